# revision 30
# baseline (speedup 1.0000x reference)
"""Trainium2 Bass kernel for nn_CausalSelfAttention_18519898980537.

Low-rank causal self-attention (12 heads, head_dim 64, only the first 16
query dims enter the scores; latent k/v dim 16/head), with the
reference's transpose-reshape scramble before the output projection.

Sharding (8 cores, SPMD single program, per-core differences are input
data only): core c = 2*b + p handles batch b with heads [6p, 6p+6) over
the full causal range.  Per query block, the normalized attention
output alT [2x96, 512] is exchanged pairwise via AllGather so both
cores of a pair see all 12 heads; each core then computes
output-feature half p of ao^T = Wvu^T @ alT, writes it to a DRAM
scratch whose flat reinterpretation is exactly the reference's
reshape(transpose(ao)) for its token half, PE-transposes the re-read
tiles, and applies its token-half of the output projection.

Optimizations vs the original baseline (471us -> ~363us):
- bf16 x/weights for the q/k/v projections (separate overlappable
  LDWEIGHTS instead of f32r self-loads; half the xT DMA), biases folded
  into the scalar-engine projection copies (per-partition bias AP).
- exp windows packed contiguously (diagonal kt pairs shifted left) so
  the scalar engine only exponentiates valid columns.
- fp8-e4m3 DoubleRow attention-value matmuls: off-diagonal kt-tile
  pairs are contracted two-at-a-time ([p,2,c] interleaved operands);
  per-head PSUM accumulator banks at partition 0 (the ISA limits
  DoubleRow destinations to col_grp quadrant 0/64); the softmax
  denominator sums the same quantized probs so fp8 bias cancels.
- heater matmul chains (full-array accumulating bf16) warm the PE_HAM
  clock gate during the initial DMA, the exposed last AllGather, and
  ahead of the output projection; low-utilization attention matmuls can
  never hold K=8/8, so the attention phase targets dense 1.2 GHz issue.
- each query block's value-up projection is deferred one full query
  block behind its AllGather so the PE queue never stalls on the
  collective; per-half normalize right after each attention half.
- bf16 tail datapath (ao scratch, scramble transposes, Wo) and the
  out-proj bias via activation; scramble split per token-half so the
  output projection overlaps the second half's transposes.
"""

import os
import sys

import numpy as np

for _p in ("/opt/trn_rl_repo", "/root/.axon_site/_ro/trn_rl_repo"):
    if os.path.isdir(_p) and _p not in sys.path:
        sys.path.insert(0, _p)

import ml_dtypes  # noqa: E402
import concourse.bacc as bacc  # noqa: E402
import concourse.mybir as mybir  # noqa: E402
from concourse.tile import TileContext  # noqa: E402
from concourse.bass_utils import run_bass_kernel_spmd  # noqa: E402

H, DH, LD, HID, LAT = 12, 64, 16, 768, 192
B, S = 4, 2048
N_CORES = 8
HL = 6  # heads per core
MHALF = HID // 2  # 384 = aoT rows per core
SCALE = LD ** -0.5

f32 = mybir.dt.float32
f32r = mybir.dt.float32r
bf16 = mybir.dt.bfloat16
f8 = mybir.dt.float8e4
AF = mybir.ActivationFunctionType
MULT = mybir.AluOpType.mult

_CACHE = {}


def _build_nc():
    if "nc" in _CACHE:
        return _CACHE["nc"]
    nc = bacc.Bacc("TRN2", target_bir_lowering=False, debug=False, num_devices=N_CORES)

    XT = nc.dram_tensor("xt", [HID + 1, S], bf16, kind="ExternalInput")
    WQ = nc.dram_tensor("wq", [HID, 256], bf16, kind="ExternalInput")
    WK = nc.dram_tensor("wk", [HID, 256], bf16, kind="ExternalInput")
    WV = nc.dram_tensor("wv", [HID + 1, 224], bf16, kind="ExternalInput")
    QKB = nc.dram_tensor("qkb", [128, 4], f32, kind="ExternalInput")
    WVU = nc.dram_tensor("wvu", [384, MHALF], f32r, kind="ExternalInput")
    WO = nc.dram_tensor("wo", [HID, HID], bf16, kind="ExternalInput")
    BO = nc.dram_tensor("bo", [128, 6], f32, kind="ExternalInput")
    MSK = nc.dram_tensor("mask", [128, 128], bf16, kind="ExternalInput")
    IDN = nc.dram_tensor("ident", [128, 128], bf16, kind="ExternalInput")
    EG = nc.dram_tensor("eg", [96, 3], f32r, kind="ExternalInput")
    EB = nc.dram_tensor("eb", [3, 96], f32r, kind="ExternalInput")
    OUTT = nc.dram_tensor("outt", [HID, S // 2], f32, kind="ExternalOutput")

    rg = [[0, 1], [2, 3], [4, 5], [6, 7]]

    with TileContext(nc) as tc:
        with (
            tc.tile_pool(name="const", bufs=1) as const,
            tc.tile_pool(name="act", bufs=1) as act,
            tc.tile_pool(name="work", bufs=1) as work,
            tc.tile_pool(name="ps", bufs=1, space="PSUM") as ps,
            tc.tile_pool(name="dram", bufs=1, space="DRAM") as dram,
        ):
            # ---- small constants first (heater depends on msk) -----------------
            msk_sb = const.tile([128, 128], bf16, name="msk")
            nc.sync.dma_start(out=msk_sb[:], in_=MSK[:, :])
            idn_sb = const.tile([128, 128], bf16, name="idn")
            nc.sync.dma_start(out=idn_sb[:], in_=IDN[:, :])
            eg_sb = const.tile([96, 3], f32r, name="eg")
            nc.sync.dma_start(out=eg_sb[:], in_=EG[:, :])
            eb_sb = const.tile([3, 96], f32r, name="eb")
            nc.sync.dma_start(out=eb_sb[:], in_=EB[:, :])
            bo_sb = const.tile([128, 6], f32, name="bo")
            nc.sync.dma_start(out=bo_sb[:], in_=BO[:, :])

            hsrc = const.tile([128, 512], bf16, name="hsrc")
            for r in range(4):
                nc.sync.dma_start(out=hsrc[:, 128 * r : 128 * r + 128], in_=MSK[:, :])

            # per-burst accumulation chain: no PSUM group resets inside a
            # burst, so heater matmuls run gap-free and the PE_HAM clock gate
            # sees a fully-busy array (promoting to K=8/8, 2.4 GHz)
            def heat(n, rhs=None):
                src = hsrc if rhs is None else rhs
                hp = ps.tile([128, 512], f32, tag="pg", bufs=1, name="heatp")
                for i in range(n):
                    nc.tensor.matmul(
                        hp[:], msk_sb[:, 0:128], src[:, 0:512],
                        start=(i == 0), stop=(i == n - 1),
                        skip_group_check=True,
                    )

            # warm the clock gate while the big xT DMA lands
            heat(42)

            # ---- weight + xT loads ---------------------------------------------
            def load_w(handle, ncols, nm, nk):
                tiles = []
                for k in range(nk):
                    pp = 128 if k < 6 else 1
                    t = const.tile(
                        [pp, ncols], bf16, name=f"{nm}{k}", tag=nm, bufs=nk,
                        padded_shape=[128, ncols],
                    )
                    nc.sync.dma_start(out=t[:], in_=handle[128 * k : 128 * k + pp, :])
                    tiles.append(t)
                return tiles

            xt_sb = []
            for k in range(7):
                pp = 128 if k < 6 else 1
                t = const.tile([pp, S], bf16, name=f"xt{k}")
                nc.sync.dma_start(out=t[:], in_=XT[128 * k : 128 * k + pp, :])
                xt_sb.append(t)

            wq_sb = load_w(WQ, 256, "wq", 6)
            wk_sb = load_w(WK, 256, "wk", 6)
            wv_sb = load_w(WV, 224, "wv", 7)
            qkb_sb = const.tile([128, 4], f32, name="qkb")
            nc.sync.dma_start(out=qkb_sb[:], in_=QKB[:, :])

            # ---- q/k projections (bf16 outputs feed the score matmuls) ---------
            qlT = [act.tile([128, S], bf16, name=f"qlT{t}") for t in range(2)]
            klT = [act.tile([128, S], bf16, name=f"klT{t}") for t in range(2)]

            def qk_proj(nb):
                # 512-token block of q and k projections; bf16 stationaries
                # use overlappable LDWEIGHTS (f32r self-loads serially); the
                # per-head bias is folded into the activation copy
                for wi, (wt, dst) in enumerate(((wq_sb, qlT), (wk_sb, klT))):
                    for t in range(2):
                        ptag = "sc" if (wi + t) % 2 == 0 else "av"
                        pt = ps.tile([128, 512], f32, tag=ptag,
                                     bufs=(2 if ptag == "sc" else 3),
                                     name="projp")
                        for k in range(6):
                            nc.tensor.matmul(
                                pt[:],
                                wt[k][:, 128 * t : 128 * t + 128],
                                xt_sb[k][:, 512 * nb : 512 * nb + 512],
                                start=(k == 0),
                                stop=(k == 5),
                            )
                        nc.scalar.activation(
                            dst[t][:, 512 * nb : 512 * nb + 512], pt[:],
                            AF.Identity, bias=qkb_sb[:, 2 * wi + t : 2 * wi + t + 1],
                        )

            vl_sb = [act.tile([128, 192], bf16, name=f"vl{i}") for i in range(16)]
            # fp8 copies packed in DoubleRow pair layout: vlp[j][:, 0:192] is
            # kt=2j, [192:384] is kt=2j+1 -> lhsT [128, 2, 32-band] slices
            vlp = [act.tile([128, 384], f8, name=f"vlp{j}") for j in range(8)]

            def vl_proj(i):
                ptag = "sc" if i % 2 == 0 else "av"
                pt = ps.tile([128, 224], f32, tag=ptag,
                             bufs=(2 if ptag == "sc" else 3), name="projv")
                for k in range(7):
                    nc.tensor.matmul(
                        pt[:],
                        xt_sb[k][:, 128 * i : 128 * i + 128],
                        wv_sb[k][:],
                        start=(k == 0),
                        stop=(k == 6),
                    )
                nc.vector.tensor_copy(vl_sb[i][:], pt[:, 0:192])
                with nc.allow_low_precision(reason="fp8 DoubleRow attention AV"):
                    nc.vector.tensor_copy(
                        vlp[i // 2][:, 192 * (i % 2) : 192 * (i % 2) + 192],
                        pt[:, 0:192],
                    )

            # ---- attention (software-pipelined over kt pairs) -------------------
            alt_t = [act.tile([96, S], f32r, name=f"alt{t}") for t in range(2)]
            cin = [dram.tile([192, 512], f32r, name=f"cin{c}") for c in range(4)]
            cout = [dram.tile([384, 512], f32r, name=f"cout{c}") for c in range(4)]
            wvu_sb = [const.tile([128, MHALF], f32r, name=f"wvu{k}") for k in range(3)]
            for k in range(3):
                nc.sync.dma_start(out=wvu_sb[k][:], in_=WVU[128 * k : 128 * k + 128, :])
            aot = dram.tile([MHALF, S], bf16, name="aot")

            def emit_attention(qb):
                for t in range(2):
                    # one accumulator bank per head, all at partition 0:
                    # DoubleRow destinations are only legal in the first PSUM
                    # quadrant group, and 32-offset bands are rejected
                    avps = [
                        ps.tile([128, 512], f32, tag="av", bufs=3, name="avp")
                        for _ in range(3)
                    ]
                    nkt = 4 * qb + 4
                    npair = nkt // 2
                    exs = {}

                    def win(j, qb=qb):
                        # packed windows: ii=1 shifted left so exp'd cols
                        # are contiguous [0, w0+w1)
                        kts = (2 * j, 2 * j + 1)
                        offs = [max(0, 128 * (kt - 4 * qb)) for kt in kts]
                        ws = [512 - o for o in offs]
                        bases = [0, ws[0]]
                        return kts, offs, ws, bases

                    def emit_scores(j, t=t, qb=qb, exs=exs):
                        kts, offs, ws, bases = win(j)
                        W = ws[0] + ws[1]
                        for g in range(3):
                            scp = ps.tile([128, 1024], f32, tag="sc", bufs=2,
                                          name="scp")
                            for ii, kt in enumerate(kts):
                                nc.tensor.matmul(
                                    scp[:, bases[ii] : bases[ii] + ws[ii]],
                                    klT[t][
                                        32 * g : 32 * g + 16,
                                        128 * kt : 128 * kt + 128,
                                    ],
                                    qlT[t][
                                        32 * g : 32 * g + 16,
                                        512 * qb + offs[ii] : 512 * qb + 512,
                                    ],
                                    start=True,
                                    stop=True,
                                    tile_position=(32 * g, 0),
                                )
                            diag = kts[1] - 4 * qb >= 0
                            ex = work.tile([128, 1024], bf16 if diag else f8,
                                           tag="expT", bufs=6, name="ex")
                            with nc.allow_low_precision(
                                reason="fp8 probs; denominator sums the same"
                                " quantized values so the ratio cancels"
                            ):
                                nc.scalar.activation(
                                    ex[:, 0:W], scp[:, 0:W], AF.Exp, bias=0.0,
                                    scale=SCALE,
                                )
                            for ii, kt in enumerate(kts):
                                if kt - 4 * qb >= 0:
                                    nc.vector.tensor_tensor(
                                        ex[:, bases[ii] : bases[ii] + 128],
                                        ex[:, bases[ii] : bases[ii] + 128],
                                        msk_sb[:],
                                        op=MULT,
                                    )
                            exs[(j, g)] = ex

                    def emit_av(j, t=t, qb=qb, nkt=nkt, avps=avps, exs=exs):
                        kts, offs, ws, bases = win(j)
                        diag = kts[1] - 4 * qb >= 0
                        for g in range(3):
                            hh = 3 * t + g
                            dst, pos = avps[g], 0
                            ex = exs.pop((j, g))
                            if not diag:
                                # both kt tiles in one fp8 DoubleRow matmul
                                nc.tensor.matmul(
                                    dst[pos : pos + 32, 0:512],
                                    vlp[j][:].rearrange(
                                        "p (two c) -> p two c", two=2
                                    )[:, :, 32 * hh : 32 * hh + 32],
                                    ex[:].rearrange(
                                        "p (two c) -> p two c", two=2
                                    ),
                                    start=(kts[0] == 0),
                                    stop=(kts[1] == nkt - 1),
                                    tile_position=(0, pos),
                                    skip_group_check=True,
                                    perf_mode=mybir.MatmulPerfMode.DoubleRow,
                                )
                                continue
                            for ii, kt in enumerate(kts):
                                nc.tensor.matmul(
                                    dst[
                                        pos : pos + 32,
                                        offs[ii] : offs[ii] + ws[ii],
                                    ],
                                    vl_sb[kt][:, 32 * hh : 32 * hh + 32],
                                    ex[:, bases[ii] : bases[ii] + ws[ii]],
                                    start=(kt == 0),
                                    stop=(kt == nkt - 1),
                                    tile_position=(0, pos),
                                    skip_group_check=True,
                                )
                            if kts[1] == nkt - 1:
                                # this head is done: drain its accumulator
                                # while the other heads' matmuls continue
                                nc.vector.tensor_copy(
                                    alt_t[t][
                                        32 * g : 32 * g + 32,
                                        512 * qb : 512 * qb + 512,
                                    ],
                                    dst[0:32, :],
                                )

                    emit_scores(0)
                    for j in range(1, npair):
                        emit_scores(j)
                        emit_av(j - 1)
                    emit_av(npair - 1)

                    normalize_half(qb, t)

            def normalize_half(qb, tt):
                # normalize + stage this half for exchange right after its
                # attention half finishes (shortens the last pre-AG chain)
                sl = slice(512 * qb, 512 * qb + 512)
                smp = ps.tile([3, 512], f32, tag="pg", bufs=1, name="smp")
                nc.tensor.matmul(
                    smp[:], eg_sb[:], alt_t[tt][:, sl], start=True, stop=True
                )
                rcp = work.tile([3, 512], f32r, tag="recip", bufs=2, name="rcp")
                with nc.allow_low_precision(reason="recip feeds fp32r matmul"):
                    nc.vector.reciprocal(rcp[:], smp[:])
                bcp = ps.tile([96, 512], f32, tag="pg", bufs=1, name="bcp")
                nc.tensor.matmul(bcp[:], eb_sb[:], rcp[:], start=True, stop=True)
                nc.vector.tensor_tensor(
                    alt_t[tt][:, sl], alt_t[tt][:, sl], bcp[:], op=MULT
                )
                nc.sync.dma_start(
                    out=cin[qb][96 * tt : 96 * tt + 96, :], in_=alt_t[tt][:, sl]
                )

            def emit_normalize(qb):
                nc.gpsimd.collective_compute(
                    "AllGather",
                    mybir.AluOpType.bypass,
                    replica_groups=rg,
                    ins=[cin[qb].opt()],
                    outs=[cout[qb].opt()],
                )

            def emit_vu(qb):
                # deferred one full query block behind the AllGather that
                # feeds it, so the PE queue never stalls on the collective
                sl = slice(512 * qb, 512 * qb + 512)
                alf = [
                    act.tile([128, 512], f32r, tag="alf", bufs=6, name=f"alf{k}")
                    for k in range(3)
                ]
                for k in range(3):
                    nc.sync.dma_start(
                        out=alf[k][:], in_=cout[qb][128 * k : 128 * k + 128, :]
                    )
                for m in range(3):
                    pt = ps.tile([128, 512], f32, tag="sc", bufs=2, name="vup",
                                 padded_shape=[128, 1024])
                    for k in range(3):
                        nc.tensor.matmul(
                            pt[:],
                            wvu_sb[k][:, 128 * m : 128 * m + 128],
                            alf[k][:],
                            start=(k == 0),
                            stop=(k == 2),
                        )
                    asb = work.tile([128, 512], bf16, tag="aosb", bufs=2, name="asb")
                    nc.scalar.activation(asb[:], pt[:], AF.Copy, bias=0.0)
                    nc.sync.dma_start(out=aot[128 * m : 128 * m + 128, sl], in_=asb[:])

            # schedule: projections upfront (dense full-array work holds the
            # clock gate at K=8); per-qb vl prefetch batches double as promote
            # bursts; each vu is deferred one query block behind its AllGather
            for nb in range(4):
                qk_proj(nb)
            for i in range(4):
                vl_proj(i)
            emit_attention(0)
            for i in range(4, 8):
                vl_proj(i)
            emit_normalize(0)
            emit_attention(1)
            for i in range(8, 12):
                vl_proj(i)
            emit_vu(0)
            emit_normalize(1)
            emit_attention(2)
            for i in range(12, 16):
                vl_proj(i)
            emit_vu(1)
            emit_normalize(2)
            emit_attention(3)
            emit_vu(2)
            emit_normalize(3)

            # ---- heater covers the last AllGather + vu drain --------------------
            heat(40, qlT[0])
            emit_vu(3)
            heat(12, qlT[0])

            # ---- scramble (flat reinterpretation) + transposes ------------------
            wo_sb = []
            for k in range(6):
                wt = const.tile([128, HID], bf16, name=f"wo{k}", tag="wo", bufs=6,
                                padded_shape=[128, HID])
                nc.sync.dma_start(out=wt[:], in_=WO[128 * k : 128 * k + 128, :])
                wo_sb.append(wt)

            aot_v = aot[:, :].rearrange("a b -> (a b)").rearrange(
                "(c d) -> c d", c=1024
            )
            # aotT[j][nbu] : scrambled-feature rows 128j..+128, token cols
            # 512*nbu..+512 -- split per nbu so the out-proj can start on the
            # first half while the second half is still transposing
            aotT = [
                [
                    act.tile([128, 512], bf16, name=f"aotT{j}_{nbu}", tag="aotT",
                             bufs=12)
                    for nbu in range(2)
                ]
                for j in range(6)
            ]
            for u in range(8):
                at = work.tile([128, HID], bf16, tag="at", bufs=2, name="at")
                nc.sync.dma_start(out=at[:], in_=aot_v[128 * u : 128 * u + 128, :])
                for j in range(6):
                    tp = ps.tile([128, 128], bf16, tag="sc", bufs=2, name="tpp")
                    nc.tensor.transpose(tp[:], at[:, 128 * j : 128 * j + 128], idn_sb[:])
                    dst = aotT[j][u // 4][:, 128 * (u % 4) : 128 * (u % 4) + 128]
                    if j % 2 == 0:
                        nc.vector.tensor_copy(dst, tp[:])
                    else:
                        nc.scalar.activation(dst, tp[:], AF.Copy, bias=0.0)

            # ---- output projection ---------------------------------------------
            heat(16, qlT[0])
            for nb in range(2):
                for m in range(6):
                    pt = ps.tile([128, 512], f32, tag="sc", bufs=2, name="outp",
                                 padded_shape=[128, 1024])
                    for k in range(6):
                        nc.tensor.matmul(
                            pt[:],
                            wo_sb[k][:, 128 * m : 128 * m + 128],
                            aotT[k][nb][:],
                            start=(k == 0),
                            stop=(k == 5),
                        )
                    osb = work.tile([128, 512], f32, tag="osb", bufs=2, name="osb")
                    nc.scalar.activation(
                        osb[:], pt[:], AF.Identity, bias=bo_sb[:, m : m + 1]
                    )
                    nc.sync.dma_start(
                        out=OUTT[128 * m : 128 * m + 128, 512 * nb : 512 * nb + 512],
                        in_=osb[:],
                    )
            # cover the final output DMA drain
            heat(12, qlT[0])

    nc.finalize()
    _CACHE["nc"] = nc
    return nc


def _host_prep(inputs):
    x = np.asarray(inputs["x"], np.float32)
    Wq = np.asarray(inputs["Wq"], np.float32)
    bq = np.asarray(inputs["bq"], np.float32)
    Wkd = np.asarray(inputs["Wkd"], np.float32)
    bkd = np.asarray(inputs["bkd"], np.float32)
    Wvd = np.asarray(inputs["Wvd"], np.float32)
    bvd = np.asarray(inputs["bvd"], np.float32)
    Wvu = np.asarray(inputs["Wvu"], np.float32)
    bvu = np.asarray(inputs["bvu"], np.float32)
    Wo = np.asarray(inputs["Wo"], np.float32)
    bo = np.asarray(inputs["bo"], np.float32)

    mask = np.tril(np.ones((128, 128), np.float32)).T.astype(ml_dtypes.bfloat16)
    ident = np.eye(128, dtype=ml_dtypes.bfloat16)
    eg = np.zeros((96, 3), np.float32)
    eb = np.zeros((3, 96), np.float32)
    for g in range(3):
        eg[32 * g + 16, g] = 1.0
        eb[g, 32 * g : 32 * g + 17] = 1.0

    bo_mat = np.ascontiguousarray(bo.reshape(6, 128).T)

    per_half = []
    for p in range(2):
        wq_pack = np.zeros((HID, 256), np.float32)
        wk_pack = np.zeros((HID, 256), np.float32)
        wv_pack = np.zeros((HID + 1, 224), np.float32)
        qkb = np.zeros((128, 4), np.float32)
        for hl in range(HL):
            hg = HL * p + hl
            t, g = hl // 3, hl % 3
            cols = slice(128 * t + 32 * g, 128 * t + 32 * g + 16)
            rows = slice(32 * g, 32 * g + 16)
            wq_pack[:, cols] = Wq[:, DH * hg : DH * hg + LD]
            qkb[rows, 0 + t] = bq[DH * hg : DH * hg + LD]
            wk_pack[:, cols] = Wkd[:, LD * hg : LD * hg + LD]
            qkb[rows, 2 + t] = bkd[LD * hg : LD * hg + LD]
            c0 = 32 * hl
            wv_pack[:HID, c0 : c0 + 16] = Wvd[:, LD * hg : LD * hg + LD]
            wv_pack[HID, c0 : c0 + 16] = bvd[LD * hg : LD * hg + LD]
            wv_pack[HID, c0 + 16] = 1.0
        wvu_pack = np.zeros((384, MHALF), np.float32)
        for hp in range(H):
            wvu_pack[32 * hp : 32 * hp + 16, :] = Wvu[
                LD * hp : LD * hp + LD, MHALF * p : MHALF * p + MHALF
            ]
        wvu_pack[16, :] = bvu[MHALF * p : MHALF * p + MHALF]
        per_half.append((
            wq_pack.astype(ml_dtypes.bfloat16),
            wk_pack.astype(ml_dtypes.bfloat16),
            wv_pack.astype(ml_dtypes.bfloat16),
            qkb, wvu_pack,
        ))

    in_maps = []
    for c in range(N_CORES):
        b, p = c // 2, c % 2
        xt = np.concatenate(
            [np.ascontiguousarray(x[b].T), np.ones((1, S), np.float32)], axis=0
        ).astype(ml_dtypes.bfloat16)
        wq_pack, wk_pack, wv_pack, qkb, wvu_pack = per_half[p]
        in_maps.append(
            dict(
                xt=xt, wq=wq_pack, wk=wk_pack, wv=wv_pack, qkb=qkb,
                wvu=wvu_pack,
                wo=Wo.astype(ml_dtypes.bfloat16), bo=bo_mat, mask=mask,
                ident=ident, eg=eg, eb=eb,
            )
        )
    return in_maps


def _run(inputs, **kw):
    nc = _build_nc()
    in_maps = _host_prep(inputs)
    return run_bass_kernel_spmd(nc, in_maps, core_ids=list(range(N_CORES)), **kw)


def kernel(**inputs):
    res = _run(inputs)
    out = np.empty((B, S, HID), np.float32)
    for b in range(B):
        for p in range(2):
            out[b, 1024 * p : 1024 * p + 1024, :] = res.results[2 * b + p]["outt"].T
    return out


# revision 31
# speedup vs baseline: 1.0207x; 1.0207x over previous
"""Trainium2 Bass kernel for nn_CausalSelfAttention_18519898980537.

Low-rank causal self-attention (12 heads, head_dim 64, only the first 16
query dims enter the scores; latent k/v dim 16/head), with the
reference's transpose-reshape scramble before the output projection.

Sharding (8 cores, SPMD single program, per-core differences are input
data only): core c = 2*b + p handles batch b with heads [6p, 6p+6) over
the full causal range.  Per query block, the normalized attention
output alT [2x96, 512] is exchanged pairwise via AllGather so both
cores of a pair see all 12 heads; each core then computes
output-feature half p of ao^T = Wvu^T @ alT, writes it to a DRAM
scratch whose flat reinterpretation is exactly the reference's
reshape(transpose(ao)) for its token half, PE-transposes the re-read
tiles, and applies its token-half of the output projection.

Optimizations vs the original baseline (471us -> ~363us):
- bf16 x/weights for the q/k/v projections (separate overlappable
  LDWEIGHTS instead of f32r self-loads; half the xT DMA), biases folded
  into the scalar-engine projection copies (per-partition bias AP).
- exp windows packed contiguously (diagonal kt pairs shifted left) so
  the scalar engine only exponentiates valid columns.
- fp8-e4m3 DoubleRow attention-value matmuls: off-diagonal kt-tile
  pairs are contracted two-at-a-time ([p,2,c] interleaved operands);
  per-head PSUM accumulator banks at partition 0 (the ISA limits
  DoubleRow destinations to col_grp quadrant 0/64); the softmax
  denominator sums the same quantized probs so fp8 bias cancels.
- heater matmul chains (full-array accumulating bf16) warm the PE_HAM
  clock gate during the initial DMA, the exposed last AllGather, and
  ahead of the output projection; low-utilization attention matmuls can
  never hold K=8/8, so the attention phase targets dense 1.2 GHz issue.
- each query block's value-up projection is deferred one full query
  block behind its AllGather so the PE queue never stalls on the
  collective; per-half normalize right after each attention half.
- bf16 tail datapath (ao scratch, scramble transposes, Wo) and the
  out-proj bias via activation; scramble split per token-half so the
  output projection overlaps the second half's transposes.
"""

import os
import sys

import numpy as np

for _p in ("/opt/trn_rl_repo", "/root/.axon_site/_ro/trn_rl_repo"):
    if os.path.isdir(_p) and _p not in sys.path:
        sys.path.insert(0, _p)

import ml_dtypes  # noqa: E402
import concourse.bacc as bacc  # noqa: E402
import concourse.mybir as mybir  # noqa: E402
from concourse.tile import TileContext  # noqa: E402
from concourse.bass_utils import run_bass_kernel_spmd  # noqa: E402

H, DH, LD, HID, LAT = 12, 64, 16, 768, 192
B, S = 4, 2048
N_CORES = 8
HL = 6  # heads per core
MHALF = HID // 2  # 384 = aoT rows per core
SCALE = LD ** -0.5

f32 = mybir.dt.float32
f32r = mybir.dt.float32r
bf16 = mybir.dt.bfloat16
f8 = mybir.dt.float8e4
AF = mybir.ActivationFunctionType
MULT = mybir.AluOpType.mult

_CACHE = {}


def _build_nc():
    if "nc" in _CACHE:
        return _CACHE["nc"]
    nc = bacc.Bacc("TRN2", target_bir_lowering=False, debug=False, num_devices=N_CORES)

    XT = nc.dram_tensor("xt", [HID + 1, S], bf16, kind="ExternalInput")
    WQ = nc.dram_tensor("wq", [HID, 256], bf16, kind="ExternalInput")
    WK = nc.dram_tensor("wk", [HID, 256], bf16, kind="ExternalInput")
    WV = nc.dram_tensor("wv", [HID + 1, 224], bf16, kind="ExternalInput")
    QKB = nc.dram_tensor("qkb", [128, 4], f32, kind="ExternalInput")
    WVU = nc.dram_tensor("wvu", [384, MHALF], f32r, kind="ExternalInput")
    WO = nc.dram_tensor("wo", [HID, HID], bf16, kind="ExternalInput")
    BO = nc.dram_tensor("bo", [128, 6], f32, kind="ExternalInput")
    MSK = nc.dram_tensor("mask", [128, 128], bf16, kind="ExternalInput")
    IDN = nc.dram_tensor("ident", [128, 128], bf16, kind="ExternalInput")
    EG = nc.dram_tensor("eg", [96, 3], f32r, kind="ExternalInput")
    EB = nc.dram_tensor("eb", [3, 96], f32r, kind="ExternalInput")
    OUTT = nc.dram_tensor("outt", [HID, S // 2], f32, kind="ExternalOutput")

    rg = [[0, 1], [2, 3], [4, 5], [6, 7]]

    with TileContext(nc) as tc:
        with (
            tc.tile_pool(name="const", bufs=1) as const,
            tc.tile_pool(name="act", bufs=1) as act,
            tc.tile_pool(name="work", bufs=1) as work,
            tc.tile_pool(name="ps", bufs=1, space="PSUM") as ps,
            tc.tile_pool(name="dram", bufs=1, space="DRAM") as dram,
        ):
            # ---- small constants first (heater depends on msk) -----------------
            msk_sb = const.tile([128, 128], bf16, name="msk")
            nc.sync.dma_start(out=msk_sb[:], in_=MSK[:, :])
            idn_sb = const.tile([128, 128], bf16, name="idn")
            nc.sync.dma_start(out=idn_sb[:], in_=IDN[:, :])
            eg_sb = const.tile([96, 3], f32r, name="eg")
            nc.sync.dma_start(out=eg_sb[:], in_=EG[:, :])
            eb_sb = const.tile([3, 96], f32r, name="eb")
            nc.sync.dma_start(out=eb_sb[:], in_=EB[:, :])
            bo_sb = const.tile([128, 6], f32, name="bo")
            nc.sync.dma_start(out=bo_sb[:], in_=BO[:, :])

            hsrc = const.tile([128, 512], bf16, name="hsrc")
            for r in range(4):
                nc.sync.dma_start(out=hsrc[:, 128 * r : 128 * r + 128], in_=MSK[:, :])

            # per-burst accumulation chain: no PSUM group resets inside a
            # burst, so heater matmuls run gap-free and the PE_HAM clock gate
            # sees a fully-busy array (promoting to K=8/8, 2.4 GHz)
            def heat(n, rhs=None):
                src = hsrc if rhs is None else rhs
                hp = ps.tile([128, 512], f32, tag="pg", bufs=1, name="heatp")
                for i in range(n):
                    nc.tensor.matmul(
                        hp[:], msk_sb[:, 0:128], src[:, 0:512],
                        start=(i == 0), stop=(i == n - 1),
                        skip_group_check=True,
                    )

            # warm the clock gate while the big xT DMA lands
            heat(30)

            # ---- weight + xT loads ---------------------------------------------
            def load_w(handle, ncols, nm, nk):
                tiles = []
                for k in range(nk):
                    pp = 128 if k < 6 else 1
                    t = const.tile(
                        [pp, ncols], bf16, name=f"{nm}{k}", tag=nm, bufs=nk,
                        padded_shape=[128, ncols],
                    )
                    nc.sync.dma_start(out=t[:], in_=handle[128 * k : 128 * k + pp, :])
                    tiles.append(t)
                return tiles

            xt_sb = []
            for k in range(7):
                pp = 128 if k < 6 else 1
                t = const.tile([pp, S], bf16, name=f"xt{k}")
                nc.sync.dma_start(out=t[:], in_=XT[128 * k : 128 * k + pp, :])
                xt_sb.append(t)

            wq_sb = load_w(WQ, 256, "wq", 6)
            wk_sb = load_w(WK, 256, "wk", 6)
            wv_sb = load_w(WV, 224, "wv", 7)
            qkb_sb = const.tile([128, 4], f32, name="qkb")
            nc.sync.dma_start(out=qkb_sb[:], in_=QKB[:, :])

            # ---- q/k projections (bf16 outputs feed the score matmuls) ---------
            qlT = [act.tile([128, S], bf16, name=f"qlT{t}") for t in range(2)]
            klT = [act.tile([128, S], bf16, name=f"klT{t}") for t in range(2)]

            def qk_proj(nb):
                # 512-token block of q and k projections; bf16 stationaries
                # use overlappable LDWEIGHTS (f32r self-loads serially); the
                # per-head bias is folded into the activation copy
                for wi, (wt, dst) in enumerate(((wq_sb, qlT), (wk_sb, klT))):
                    for t in range(2):
                        ptag = "sc" if (wi + t) % 2 == 0 else "av"
                        pt = ps.tile([128, 512], f32, tag=ptag,
                                     bufs=(2 if ptag == "sc" else 3),
                                     name="projp")
                        for k in range(6):
                            nc.tensor.matmul(
                                pt[:],
                                wt[k][:, 128 * t : 128 * t + 128],
                                xt_sb[k][:, 512 * nb : 512 * nb + 512],
                                start=(k == 0),
                                stop=(k == 5),
                            )
                        nc.scalar.activation(
                            dst[t][:, 512 * nb : 512 * nb + 512], pt[:],
                            AF.Identity, bias=qkb_sb[:, 2 * wi + t : 2 * wi + t + 1],
                        )

            vl_sb = [act.tile([128, 192], bf16, name=f"vl{i}") for i in range(16)]
            # fp8 copies packed in DoubleRow pair layout: vlp[j][:, 0:192] is
            # kt=2j, [192:384] is kt=2j+1 -> lhsT [128, 2, 32-band] slices
            vlp = [act.tile([128, 384], f8, name=f"vlp{j}") for j in range(8)]

            def vl_proj(i):
                ptag = "sc" if i % 2 == 0 else "av"
                pt = ps.tile([128, 224], f32, tag=ptag,
                             bufs=(2 if ptag == "sc" else 3), name="projv")
                for k in range(7):
                    nc.tensor.matmul(
                        pt[:],
                        xt_sb[k][:, 128 * i : 128 * i + 128],
                        wv_sb[k][:],
                        start=(k == 0),
                        stop=(k == 6),
                    )
                nc.vector.tensor_copy(vl_sb[i][:], pt[:, 0:192])
                with nc.allow_low_precision(reason="fp8 DoubleRow attention AV"):
                    nc.vector.tensor_copy(
                        vlp[i // 2][:, 192 * (i % 2) : 192 * (i % 2) + 192],
                        pt[:, 0:192],
                    )

            # ---- attention (software-pipelined over kt pairs) -------------------
            alt_t = [act.tile([96, S], f32r, name=f"alt{t}") for t in range(2)]
            cin = [dram.tile([192, 512], f32r, name=f"cin{c}") for c in range(4)]
            cout = [dram.tile([384, 512], f32r, name=f"cout{c}") for c in range(4)]
            wvu_sb = [const.tile([128, MHALF], f32r, name=f"wvu{k}") for k in range(3)]
            for k in range(3):
                nc.sync.dma_start(out=wvu_sb[k][:], in_=WVU[128 * k : 128 * k + 128, :])
            aot = dram.tile([MHALF, S], bf16, name="aot")

            def emit_attention(qb):
                for t in range(2):
                    # one accumulator bank per head, all at partition 0:
                    # DoubleRow destinations are only legal in the first PSUM
                    # quadrant group, and 32-offset bands are rejected
                    avps = [
                        ps.tile([128, 512], f32, tag="av", bufs=3, name="avp")
                        for _ in range(3)
                    ]
                    nkt = 4 * qb + 4
                    npair = nkt // 2
                    exs = {}

                    def win(j, qb=qb):
                        # packed windows: ii=1 shifted left so exp'd cols
                        # are contiguous [0, w0+w1)
                        kts = (2 * j, 2 * j + 1)
                        offs = [max(0, 128 * (kt - 4 * qb)) for kt in kts]
                        ws = [512 - o for o in offs]
                        bases = [0, ws[0]]
                        return kts, offs, ws, bases

                    def emit_scores(j, t=t, qb=qb, exs=exs):
                        kts, offs, ws, bases = win(j)
                        W = ws[0] + ws[1]
                        for g in range(3):
                            scp = ps.tile([128, 1024], f32, tag="sc", bufs=2,
                                          name="scp")
                            for ii, kt in enumerate(kts):
                                nc.tensor.matmul(
                                    scp[:, bases[ii] : bases[ii] + ws[ii]],
                                    klT[t][
                                        32 * g : 32 * g + 16,
                                        128 * kt : 128 * kt + 128,
                                    ],
                                    qlT[t][
                                        32 * g : 32 * g + 16,
                                        512 * qb + offs[ii] : 512 * qb + 512,
                                    ],
                                    start=True,
                                    stop=True,
                                    tile_position=(32 * g, 0),
                                )
                            diag = kts[1] - 4 * qb >= 0
                            ex = work.tile([128, 1024], bf16 if diag else f8,
                                           tag="expT", bufs=6, name="ex")
                            with nc.allow_low_precision(
                                reason="fp8 probs; denominator sums the same"
                                " quantized values so the ratio cancels"
                            ):
                                nc.scalar.activation(
                                    ex[:, 0:W], scp[:, 0:W], AF.Exp, bias=0.0,
                                    scale=SCALE,
                                )
                            for ii, kt in enumerate(kts):
                                if kt - 4 * qb >= 0:
                                    nc.vector.tensor_tensor(
                                        ex[:, bases[ii] : bases[ii] + 128],
                                        ex[:, bases[ii] : bases[ii] + 128],
                                        msk_sb[:],
                                        op=MULT,
                                    )
                            exs[(j, g)] = ex

                    def emit_av(j, t=t, qb=qb, nkt=nkt, avps=avps, exs=exs):
                        kts, offs, ws, bases = win(j)
                        diag = kts[1] - 4 * qb >= 0
                        for g in range(3):
                            hh = 3 * t + g
                            dst, pos = avps[g], 0
                            ex = exs.pop((j, g))
                            if not diag:
                                # both kt tiles in one fp8 DoubleRow matmul
                                nc.tensor.matmul(
                                    dst[pos : pos + 32, 0:512],
                                    vlp[j][:].rearrange(
                                        "p (two c) -> p two c", two=2
                                    )[:, :, 32 * hh : 32 * hh + 32],
                                    ex[:].rearrange(
                                        "p (two c) -> p two c", two=2
                                    ),
                                    start=(kts[0] == 0),
                                    stop=(kts[1] == nkt - 1),
                                    tile_position=(0, pos),
                                    skip_group_check=True,
                                    perf_mode=mybir.MatmulPerfMode.DoubleRow,
                                )
                                continue
                            for ii, kt in enumerate(kts):
                                nc.tensor.matmul(
                                    dst[
                                        pos : pos + 32,
                                        offs[ii] : offs[ii] + ws[ii],
                                    ],
                                    vl_sb[kt][:, 32 * hh : 32 * hh + 32],
                                    ex[:, bases[ii] : bases[ii] + ws[ii]],
                                    start=(kt == 0),
                                    stop=(kt == nkt - 1),
                                    tile_position=(0, pos),
                                    skip_group_check=True,
                                )
                            if kts[1] == nkt - 1:
                                # this head is done: drain its accumulator
                                # while the other heads' matmuls continue
                                nc.vector.tensor_copy(
                                    alt_t[t][
                                        32 * g : 32 * g + 32,
                                        512 * qb : 512 * qb + 512,
                                    ],
                                    dst[0:32, :],
                                )

                    emit_scores(0)
                    for j in range(1, npair):
                        emit_scores(j)
                        emit_av(j - 1)
                    emit_av(npair - 1)

                    normalize_half(qb, t)

            def normalize_half(qb, tt):
                # normalize + stage this half for exchange right after its
                # attention half finishes (shortens the last pre-AG chain)
                sl = slice(512 * qb, 512 * qb + 512)
                smp = ps.tile([3, 512], f32, tag="pg", bufs=1, name="smp")
                nc.tensor.matmul(
                    smp[:], eg_sb[:], alt_t[tt][:, sl], start=True, stop=True
                )
                rcp = work.tile([3, 512], f32r, tag="recip", bufs=2, name="rcp")
                with nc.allow_low_precision(reason="recip feeds fp32r matmul"):
                    nc.vector.reciprocal(rcp[:], smp[:])
                bcp = ps.tile([96, 512], f32, tag="pg", bufs=1, name="bcp")
                nc.tensor.matmul(bcp[:], eb_sb[:], rcp[:], start=True, stop=True)
                nc.vector.tensor_tensor(
                    alt_t[tt][:, sl], alt_t[tt][:, sl], bcp[:], op=MULT
                )
                nc.sync.dma_start(
                    out=cin[qb][96 * tt : 96 * tt + 96, :], in_=alt_t[tt][:, sl]
                )

            def emit_normalize(qb):
                nc.gpsimd.collective_compute(
                    "AllGather",
                    mybir.AluOpType.bypass,
                    replica_groups=rg,
                    ins=[cin[qb].opt()],
                    outs=[cout[qb].opt()],
                )

            def emit_vu(qb):
                # deferred one full query block behind the AllGather that
                # feeds it, so the PE queue never stalls on the collective
                sl = slice(512 * qb, 512 * qb + 512)
                alf = [
                    act.tile([128, 512], f32r, tag="alf", bufs=6, name=f"alf{k}")
                    for k in range(3)
                ]
                for k in range(3):
                    nc.sync.dma_start(
                        out=alf[k][:], in_=cout[qb][128 * k : 128 * k + 128, :]
                    )
                for m in range(3):
                    pt = ps.tile([128, 512], f32, tag="sc", bufs=2, name="vup",
                                 padded_shape=[128, 1024])
                    for k in range(3):
                        nc.tensor.matmul(
                            pt[:],
                            wvu_sb[k][:, 128 * m : 128 * m + 128],
                            alf[k][:],
                            start=(k == 0),
                            stop=(k == 2),
                        )
                    asb = work.tile([128, 512], bf16, tag="aosb", bufs=2, name="asb")
                    nc.scalar.activation(asb[:], pt[:], AF.Copy, bias=0.0)
                    nc.sync.dma_start(out=aot[128 * m : 128 * m + 128, sl], in_=asb[:])

            # schedule: projections upfront (dense full-array work holds the
            # clock gate at K=8); per-qb vl prefetch batches double as promote
            # bursts; each vu is deferred one query block behind its AllGather
            for nb in range(4):
                qk_proj(nb)
            for i in range(4):
                vl_proj(i)
            emit_attention(0)
            for i in range(4, 8):
                vl_proj(i)
            emit_normalize(0)
            emit_attention(1)
            for i in range(8, 12):
                vl_proj(i)
            emit_vu(0)
            emit_normalize(1)
            emit_attention(2)
            for i in range(12, 16):
                vl_proj(i)
            emit_vu(1)
            emit_normalize(2)
            emit_attention(3)
            emit_vu(2)
            emit_normalize(3)

            # ---- heater covers the last AllGather + vu drain --------------------
            heat(40, qlT[0])
            emit_vu(3)
            heat(12, qlT[0])

            # ---- scramble (flat reinterpretation) + transposes ------------------
            wo_sb = []
            for k in range(6):
                wt = const.tile([128, HID], bf16, name=f"wo{k}", tag="wo", bufs=6,
                                padded_shape=[128, HID])
                nc.sync.dma_start(out=wt[:], in_=WO[128 * k : 128 * k + 128, :])
                wo_sb.append(wt)

            aot_v = aot[:, :].rearrange("a b -> (a b)").rearrange(
                "(c d) -> c d", c=1024
            )
            # aotT[j][nbu] : scrambled-feature rows 128j..+128, token cols
            # 512*nbu..+512 -- split per nbu so the out-proj can start on the
            # first half while the second half is still transposing
            aotT = [
                [
                    act.tile([128, 512], bf16, name=f"aotT{j}_{nbu}", tag="aotT",
                             bufs=12)
                    for nbu in range(2)
                ]
                for j in range(6)
            ]
            for u in range(8):
                at = work.tile([128, HID], bf16, tag="at", bufs=3, name="at")
                nc.sync.dma_start(out=at[:], in_=aot_v[128 * u : 128 * u + 128, :])
                for j in range(6):
                    ptag = "sc" if j % 2 == 0 else "av"
                    tp = ps.tile([128, 128], bf16, tag=ptag,
                                 bufs=(2 if ptag == "sc" else 3), name="tpp")
                    nc.tensor.transpose(tp[:], at[:, 128 * j : 128 * j + 128], idn_sb[:])
                    dst = aotT[j][u // 4][:, 128 * (u % 4) : 128 * (u % 4) + 128]
                    if j % 2 == 0:
                        nc.vector.tensor_copy(dst, tp[:])
                    else:
                        nc.scalar.activation(dst, tp[:], AF.Copy, bias=0.0)

            # ---- output projection ---------------------------------------------
            heat(16, qlT[0])
            for nb in range(2):
                for m in range(6):
                    pt = ps.tile([128, 512], f32, tag="sc", bufs=2, name="outp",
                                 padded_shape=[128, 1024])
                    for k in range(6):
                        nc.tensor.matmul(
                            pt[:],
                            wo_sb[k][:, 128 * m : 128 * m + 128],
                            aotT[k][nb][:],
                            start=(k == 0),
                            stop=(k == 5),
                        )
                    osb = work.tile([128, 512], f32, tag="osb", bufs=2, name="osb")
                    nc.scalar.activation(
                        osb[:], pt[:], AF.Identity, bias=bo_sb[:, m : m + 1]
                    )
                    nc.sync.dma_start(
                        out=OUTT[128 * m : 128 * m + 128, 512 * nb : 512 * nb + 512],
                        in_=osb[:],
                    )
            # cover the final output DMA drain
            heat(12, qlT[0])

    nc.finalize()
    _CACHE["nc"] = nc
    return nc


def _host_prep(inputs):
    x = np.asarray(inputs["x"], np.float32)
    Wq = np.asarray(inputs["Wq"], np.float32)
    bq = np.asarray(inputs["bq"], np.float32)
    Wkd = np.asarray(inputs["Wkd"], np.float32)
    bkd = np.asarray(inputs["bkd"], np.float32)
    Wvd = np.asarray(inputs["Wvd"], np.float32)
    bvd = np.asarray(inputs["bvd"], np.float32)
    Wvu = np.asarray(inputs["Wvu"], np.float32)
    bvu = np.asarray(inputs["bvu"], np.float32)
    Wo = np.asarray(inputs["Wo"], np.float32)
    bo = np.asarray(inputs["bo"], np.float32)

    mask = np.tril(np.ones((128, 128), np.float32)).T.astype(ml_dtypes.bfloat16)
    ident = np.eye(128, dtype=ml_dtypes.bfloat16)
    eg = np.zeros((96, 3), np.float32)
    eb = np.zeros((3, 96), np.float32)
    for g in range(3):
        eg[32 * g + 16, g] = 1.0
        eb[g, 32 * g : 32 * g + 17] = 1.0

    bo_mat = np.ascontiguousarray(bo.reshape(6, 128).T)

    per_half = []
    for p in range(2):
        wq_pack = np.zeros((HID, 256), np.float32)
        wk_pack = np.zeros((HID, 256), np.float32)
        wv_pack = np.zeros((HID + 1, 224), np.float32)
        qkb = np.zeros((128, 4), np.float32)
        for hl in range(HL):
            hg = HL * p + hl
            t, g = hl // 3, hl % 3
            cols = slice(128 * t + 32 * g, 128 * t + 32 * g + 16)
            rows = slice(32 * g, 32 * g + 16)
            wq_pack[:, cols] = Wq[:, DH * hg : DH * hg + LD]
            qkb[rows, 0 + t] = bq[DH * hg : DH * hg + LD]
            wk_pack[:, cols] = Wkd[:, LD * hg : LD * hg + LD]
            qkb[rows, 2 + t] = bkd[LD * hg : LD * hg + LD]
            c0 = 32 * hl
            wv_pack[:HID, c0 : c0 + 16] = Wvd[:, LD * hg : LD * hg + LD]
            wv_pack[HID, c0 : c0 + 16] = bvd[LD * hg : LD * hg + LD]
            wv_pack[HID, c0 + 16] = 1.0
        wvu_pack = np.zeros((384, MHALF), np.float32)
        for hp in range(H):
            wvu_pack[32 * hp : 32 * hp + 16, :] = Wvu[
                LD * hp : LD * hp + LD, MHALF * p : MHALF * p + MHALF
            ]
        wvu_pack[16, :] = bvu[MHALF * p : MHALF * p + MHALF]
        per_half.append((
            wq_pack.astype(ml_dtypes.bfloat16),
            wk_pack.astype(ml_dtypes.bfloat16),
            wv_pack.astype(ml_dtypes.bfloat16),
            qkb, wvu_pack,
        ))

    in_maps = []
    for c in range(N_CORES):
        b, p = c // 2, c % 2
        xt = np.concatenate(
            [np.ascontiguousarray(x[b].T), np.ones((1, S), np.float32)], axis=0
        ).astype(ml_dtypes.bfloat16)
        wq_pack, wk_pack, wv_pack, qkb, wvu_pack = per_half[p]
        in_maps.append(
            dict(
                xt=xt, wq=wq_pack, wk=wk_pack, wv=wv_pack, qkb=qkb,
                wvu=wvu_pack,
                wo=Wo.astype(ml_dtypes.bfloat16), bo=bo_mat, mask=mask,
                ident=ident, eg=eg, eb=eb,
            )
        )
    return in_maps


def _run(inputs, **kw):
    nc = _build_nc()
    in_maps = _host_prep(inputs)
    return run_bass_kernel_spmd(nc, in_maps, core_ids=list(range(N_CORES)), **kw)


def kernel(**inputs):
    res = _run(inputs)
    out = np.empty((B, S, HID), np.float32)
    for b in range(B):
        for p in range(2):
            out[b, 1024 * p : 1024 * p + 1024, :] = res.results[2 * b + p]["outt"].T
    return out


# revision 33
# speedup vs baseline: 1.0517x; 1.0304x over previous
"""Trainium2 Bass kernel for nn_CausalSelfAttention_18519898980537.

Low-rank causal self-attention (12 heads, head_dim 64, only the first 16
query dims enter the scores; latent k/v dim 16/head), with the
reference's transpose-reshape scramble before the output projection.

Sharding (8 cores, SPMD single program, per-core differences are input
data only): core c = 2*b + p handles batch b with heads [6p, 6p+6) over
the full causal range.  Per query block, the normalized attention
output alT [2x96, 512] is exchanged pairwise via AllGather so both
cores of a pair see all 12 heads; each core then computes
output-feature half p of ao^T = Wvu^T @ alT, writes it to a DRAM
scratch whose flat reinterpretation is exactly the reference's
reshape(transpose(ao)) for its token half, PE-transposes the re-read
tiles, and applies its token-half of the output projection.

Optimizations vs the original baseline (471us -> ~355us):
- bf16 x/weights for the q/k/v projections (separate overlappable
  LDWEIGHTS instead of f32r self-loads; half the xT DMA), biases folded
  into the scalar-engine projection copies (per-partition bias AP).
- exp windows packed contiguously (diagonal kt pairs shifted left) so
  the scalar engine only exponentiates valid columns.
- fp8-e4m3 DoubleRow attention-value matmuls: off-diagonal kt-tile
  pairs are contracted two-at-a-time ([p,2,c] interleaved operands);
  per-head PSUM accumulator banks at partition 0 (the ISA limits
  DoubleRow destinations to col_grp quadrant 0/64); the softmax
  denominator sums the same quantized probs so fp8 bias cancels.
- heater matmul chains (full-array accumulating bf16) warm the PE_HAM
  clock gate during the initial DMA, the exposed last AllGather, and
  ahead of the output projection; low-utilization attention matmuls can
  never hold K=8/8, so the attention phase targets dense 1.2 GHz issue.
- each query block's value-up projection is deferred one full query
  block behind its AllGather so the PE queue never stalls on the
  collective; per-half normalize right after each attention half.
- bf16 tail datapath (ao scratch, scramble transposes, Wo) and the
  out-proj bias via activation; scramble split per token-half so the
  output projection overlaps the second half's transposes.
"""

import os
import sys

import numpy as np

for _p in ("/opt/trn_rl_repo", "/root/.axon_site/_ro/trn_rl_repo"):
    if os.path.isdir(_p) and _p not in sys.path:
        sys.path.insert(0, _p)

import ml_dtypes  # noqa: E402
import concourse.bacc as bacc  # noqa: E402
import concourse.mybir as mybir  # noqa: E402
from concourse.tile import TileContext  # noqa: E402
from concourse.bass_utils import run_bass_kernel_spmd  # noqa: E402

H, DH, LD, HID, LAT = 12, 64, 16, 768, 192
B, S = 4, 2048
N_CORES = 8
HL = 6  # heads per core
MHALF = HID // 2  # 384 = aoT rows per core
SCALE = LD ** -0.5

f32 = mybir.dt.float32
f32r = mybir.dt.float32r
bf16 = mybir.dt.bfloat16
f8 = mybir.dt.float8e4
AF = mybir.ActivationFunctionType
MULT = mybir.AluOpType.mult

_CACHE = {}


def _build_nc():
    if "nc" in _CACHE:
        return _CACHE["nc"]
    nc = bacc.Bacc("TRN2", target_bir_lowering=False, debug=False, num_devices=N_CORES)

    XT = nc.dram_tensor("xt", [HID + 1, S], bf16, kind="ExternalInput")
    WQ = nc.dram_tensor("wq", [HID, 256], bf16, kind="ExternalInput")
    WK = nc.dram_tensor("wk", [HID, 256], bf16, kind="ExternalInput")
    WV = nc.dram_tensor("wv", [HID + 1, 224], bf16, kind="ExternalInput")
    QKB = nc.dram_tensor("qkb", [128, 4], f32, kind="ExternalInput")
    WVU = nc.dram_tensor("wvu", [384, MHALF], f32r, kind="ExternalInput")
    WO = nc.dram_tensor("wo", [HID, HID], bf16, kind="ExternalInput")
    BO = nc.dram_tensor("bo", [128, 6], f32, kind="ExternalInput")
    MSK = nc.dram_tensor("mask", [128, 128], bf16, kind="ExternalInput")
    IDN = nc.dram_tensor("ident", [128, 128], bf16, kind="ExternalInput")
    EG = nc.dram_tensor("eg", [96, 3], f32r, kind="ExternalInput")
    EB = nc.dram_tensor("eb", [3, 96], f32r, kind="ExternalInput")
    OUTT = nc.dram_tensor("outt", [HID, S // 2], f32, kind="ExternalOutput")

    rg = [[0, 1], [2, 3], [4, 5], [6, 7]]

    with TileContext(nc) as tc:
        with (
            tc.tile_pool(name="const", bufs=1) as const,
            tc.tile_pool(name="act", bufs=1) as act,
            tc.tile_pool(name="work", bufs=1) as work,
            tc.tile_pool(name="ps", bufs=1, space="PSUM") as ps,
            tc.tile_pool(name="dram", bufs=1, space="DRAM") as dram,
        ):
            # ---- small constants first (heater depends on msk) -----------------
            msk_sb = const.tile([128, 128], bf16, name="msk")
            nc.sync.dma_start(out=msk_sb[:], in_=MSK[:, :])
            idn_sb = const.tile([128, 128], bf16, name="idn")
            nc.sync.dma_start(out=idn_sb[:], in_=IDN[:, :])
            eg_sb = const.tile([96, 3], f32r, name="eg")
            nc.sync.dma_start(out=eg_sb[:], in_=EG[:, :])
            eb_sb = const.tile([3, 96], f32r, name="eb")
            nc.sync.dma_start(out=eb_sb[:], in_=EB[:, :])
            bo_sb = const.tile([128, 6], f32, name="bo")
            nc.sync.dma_start(out=bo_sb[:], in_=BO[:, :])

            hsrc = const.tile([128, 512], bf16, name="hsrc")
            for r in range(4):
                nc.sync.dma_start(out=hsrc[:, 128 * r : 128 * r + 128], in_=MSK[:, :])

            # per-burst accumulation chain: no PSUM group resets inside a
            # burst, so heater matmuls run gap-free and the PE_HAM clock gate
            # sees a fully-busy array (promoting to K=8/8, 2.4 GHz)
            def heat(n, rhs=None):
                src = hsrc if rhs is None else rhs
                hp = ps.tile([128, 512], f32, tag="pg", bufs=1, name="heatp")
                for i in range(n):
                    nc.tensor.matmul(
                        hp[:], msk_sb[:, 0:128], src[:, 0:512],
                        start=(i == 0), stop=(i == n - 1),
                        skip_group_check=True,
                    )

            # warm the clock gate while the big xT DMA lands
            heat(30)

            # ---- weight + xT loads ---------------------------------------------
            def load_w(handle, ncols, nm, nk):
                tiles = []
                for k in range(nk):
                    pp = 128 if k < 6 else 1
                    t = const.tile(
                        [pp, ncols], bf16, name=f"{nm}{k}", tag=nm, bufs=nk,
                        padded_shape=[128, ncols],
                    )
                    nc.sync.dma_start(out=t[:], in_=handle[128 * k : 128 * k + pp, :])
                    tiles.append(t)
                return tiles

            xt_sb = []
            for k in range(7):
                pp = 128 if k < 6 else 1
                t = const.tile([pp, S], bf16, name=f"xt{k}")
                for hh in range(2):
                    nc.sync.dma_start(
                        out=t[:, 1024 * hh : 1024 * hh + 1024],
                        in_=XT[128 * k : 128 * k + pp, 1024 * hh : 1024 * hh + 1024],
                    )
                xt_sb.append(t)

            wq_sb = load_w(WQ, 256, "wq", 6)
            wk_sb = load_w(WK, 256, "wk", 6)
            wv_sb = load_w(WV, 224, "wv", 7)
            qkb_sb = const.tile([128, 4], f32, name="qkb")
            nc.sync.dma_start(out=qkb_sb[:], in_=QKB[:, :])

            # ---- q/k projections (bf16 outputs feed the score matmuls) ---------
            qlT = [act.tile([128, S], bf16, name=f"qlT{t}") for t in range(2)]
            klT = [act.tile([128, S], bf16, name=f"klT{t}") for t in range(2)]

            def qk_proj(nb):
                # 512-token block of q and k projections; bf16 stationaries
                # use overlappable LDWEIGHTS (f32r self-loads serially); the
                # per-head bias is folded into the activation copy
                for wi, (wt, dst) in enumerate(((wq_sb, qlT), (wk_sb, klT))):
                    for t in range(2):
                        ptag = "sc" if (wi + t) % 2 == 0 else "av"
                        pt = ps.tile([128, 512], f32, tag=ptag,
                                     bufs=(2 if ptag == "sc" else 3),
                                     name="projp")
                        for k in range(6):
                            nc.tensor.matmul(
                                pt[:],
                                wt[k][:, 128 * t : 128 * t + 128],
                                xt_sb[k][:, 512 * nb : 512 * nb + 512],
                                start=(k == 0),
                                stop=(k == 5),
                            )
                        nc.scalar.activation(
                            dst[t][:, 512 * nb : 512 * nb + 512], pt[:],
                            AF.Identity, bias=qkb_sb[:, 2 * wi + t : 2 * wi + t + 1],
                        )

            vl_sb = [act.tile([128, 192], bf16, name=f"vl{i}") for i in range(16)]
            # fp8 copies packed in DoubleRow pair layout: vlp[j][:, 0:192] is
            # kt=2j, [192:384] is kt=2j+1 -> lhsT [128, 2, 32-band] slices
            vlp = [act.tile([128, 384], f8, name=f"vlp{j}") for j in range(8)]

            def vl_proj(i):
                ptag = "sc" if i % 2 == 0 else "av"
                pt = ps.tile([128, 224], f32, tag=ptag,
                             bufs=(2 if ptag == "sc" else 3), name="projv")
                for k in range(7):
                    nc.tensor.matmul(
                        pt[:],
                        xt_sb[k][:, 128 * i : 128 * i + 128],
                        wv_sb[k][:],
                        start=(k == 0),
                        stop=(k == 6),
                    )
                nc.vector.tensor_copy(vl_sb[i][:], pt[:, 0:192])
                with nc.allow_low_precision(reason="fp8 DoubleRow attention AV"):
                    nc.vector.tensor_copy(
                        vlp[i // 2][:, 192 * (i % 2) : 192 * (i % 2) + 192],
                        pt[:, 0:192],
                    )

            # ---- attention (software-pipelined over kt pairs) -------------------
            alt_t = [act.tile([96, S], f32r, name=f"alt{t}") for t in range(2)]
            cin = [dram.tile([192, 512], f32r, name=f"cin{c}") for c in range(4)]
            cout = [dram.tile([384, 512], f32r, name=f"cout{c}") for c in range(4)]
            wvu_sb = [const.tile([128, MHALF], f32r, name=f"wvu{k}") for k in range(3)]
            for k in range(3):
                nc.sync.dma_start(out=wvu_sb[k][:], in_=WVU[128 * k : 128 * k + 128, :])
            aot = dram.tile([MHALF, S], bf16, name="aot")

            def emit_attention(qb):
                for t in range(2):
                    # one accumulator bank per head, all at partition 0:
                    # DoubleRow destinations are only legal in the first PSUM
                    # quadrant group, and 32-offset bands are rejected
                    avps = [
                        ps.tile([128, 512], f32, tag="av", bufs=3, name="avp")
                        for _ in range(3)
                    ]
                    nkt = 4 * qb + 4
                    npair = nkt // 2
                    exs = {}

                    def win(j, qb=qb):
                        # packed windows: ii=1 shifted left so exp'd cols
                        # are contiguous [0, w0+w1)
                        kts = (2 * j, 2 * j + 1)
                        offs = [max(0, 128 * (kt - 4 * qb)) for kt in kts]
                        ws = [512 - o for o in offs]
                        bases = [0, ws[0]]
                        return kts, offs, ws, bases

                    def emit_scores(j, t=t, qb=qb, exs=exs):
                        kts, offs, ws, bases = win(j)
                        W = ws[0] + ws[1]
                        for g in range(3):
                            scp = ps.tile([128, 1024], f32, tag="sc", bufs=2,
                                          name="scp")
                            for ii, kt in enumerate(kts):
                                nc.tensor.matmul(
                                    scp[:, bases[ii] : bases[ii] + ws[ii]],
                                    klT[t][
                                        32 * g : 32 * g + 16,
                                        128 * kt : 128 * kt + 128,
                                    ],
                                    qlT[t][
                                        32 * g : 32 * g + 16,
                                        512 * qb + offs[ii] : 512 * qb + 512,
                                    ],
                                    start=True,
                                    stop=True,
                                    tile_position=(32 * g, 0),
                                )
                            diag = kts[1] - 4 * qb >= 0
                            ex = work.tile([128, 1024], bf16 if diag else f8,
                                           tag="expT", bufs=6, name="ex")
                            with nc.allow_low_precision(
                                reason="fp8 probs; denominator sums the same"
                                " quantized values so the ratio cancels"
                            ):
                                nc.scalar.activation(
                                    ex[:, 0:W], scp[:, 0:W], AF.Exp, bias=0.0,
                                    scale=SCALE,
                                )
                            for ii, kt in enumerate(kts):
                                if kt - 4 * qb >= 0:
                                    nc.vector.tensor_tensor(
                                        ex[:, bases[ii] : bases[ii] + 128],
                                        ex[:, bases[ii] : bases[ii] + 128],
                                        msk_sb[:],
                                        op=MULT,
                                    )
                            exs[(j, g)] = ex

                    def emit_av(j, t=t, qb=qb, nkt=nkt, avps=avps, exs=exs):
                        kts, offs, ws, bases = win(j)
                        diag = kts[1] - 4 * qb >= 0
                        for g in range(3):
                            hh = 3 * t + g
                            dst, pos = avps[g], 0
                            ex = exs.pop((j, g))
                            if not diag:
                                # both kt tiles in one fp8 DoubleRow matmul
                                nc.tensor.matmul(
                                    dst[pos : pos + 32, 0:512],
                                    vlp[j][:].rearrange(
                                        "p (two c) -> p two c", two=2
                                    )[:, :, 32 * hh : 32 * hh + 32],
                                    ex[:].rearrange(
                                        "p (two c) -> p two c", two=2
                                    ),
                                    start=(kts[0] == 0),
                                    stop=(kts[1] == nkt - 1),
                                    tile_position=(0, pos),
                                    skip_group_check=True,
                                    perf_mode=mybir.MatmulPerfMode.DoubleRow,
                                )
                                continue
                            for ii, kt in enumerate(kts):
                                nc.tensor.matmul(
                                    dst[
                                        pos : pos + 32,
                                        offs[ii] : offs[ii] + ws[ii],
                                    ],
                                    vl_sb[kt][:, 32 * hh : 32 * hh + 32],
                                    ex[:, bases[ii] : bases[ii] + ws[ii]],
                                    start=(kt == 0),
                                    stop=(kt == nkt - 1),
                                    tile_position=(0, pos),
                                    skip_group_check=True,
                                )
                            if kts[1] == nkt - 1:
                                # this head is done: drain its accumulator
                                # while the other heads' matmuls continue
                                nc.vector.tensor_copy(
                                    alt_t[t][
                                        32 * g : 32 * g + 32,
                                        512 * qb : 512 * qb + 512,
                                    ],
                                    dst[0:32, :],
                                )

                    emit_scores(0)
                    for j in range(1, npair):
                        emit_scores(j)
                        if j == 1:
                            # the previous half's normalize goes here: its smp
                            # matmul then reaches the PE queue head well after
                            # the DVE accumulator drains it depends on
                            while pending_norm:
                                normalize_half(*pending_norm.pop(0))
                        emit_av(j - 1)
                    emit_av(npair - 1)

                    pending_norm.append((qb, t))

            def normalize_half(qb, tt):
                # normalize + stage this half for exchange right after its
                # attention half finishes (shortens the last pre-AG chain)
                sl = slice(512 * qb, 512 * qb + 512)
                smp = ps.tile([3, 512], f32, tag="pg", bufs=1, name="smp")
                nc.tensor.matmul(
                    smp[:], eg_sb[:], alt_t[tt][:, sl], start=True, stop=True
                )
                rcp = work.tile([3, 512], f32r, tag="recip", bufs=2, name="rcp")
                with nc.allow_low_precision(reason="recip feeds fp32r matmul"):
                    nc.vector.reciprocal(rcp[:], smp[:])
                bcp = ps.tile([96, 512], f32, tag="pg", bufs=1, name="bcp")
                nc.tensor.matmul(bcp[:], eb_sb[:], rcp[:], start=True, stop=True)
                nc.vector.tensor_tensor(
                    alt_t[tt][:, sl], alt_t[tt][:, sl], bcp[:], op=MULT
                )
                nc.sync.dma_start(
                    out=cin[qb][96 * tt : 96 * tt + 96, :], in_=alt_t[tt][:, sl]
                )

            pending_norm = []

            def emit_normalize(qb):
                while pending_norm:
                    normalize_half(*pending_norm.pop(0))
                nc.gpsimd.collective_compute(
                    "AllGather",
                    mybir.AluOpType.bypass,
                    replica_groups=rg,
                    ins=[cin[qb].opt()],
                    outs=[cout[qb].opt()],
                )

            def emit_vu(qb):
                # deferred one full query block behind the AllGather that
                # feeds it, so the PE queue never stalls on the collective
                sl = slice(512 * qb, 512 * qb + 512)
                alf = [
                    act.tile([128, 512], f32r, tag="alf", bufs=6, name=f"alf{k}")
                    for k in range(3)
                ]
                for k in range(3):
                    nc.sync.dma_start(
                        out=alf[k][:], in_=cout[qb][128 * k : 128 * k + 128, :]
                    )
                for m in range(3):
                    pt = ps.tile([128, 512], f32, tag="sc", bufs=2, name="vup",
                                 padded_shape=[128, 1024])
                    for k in range(3):
                        nc.tensor.matmul(
                            pt[:],
                            wvu_sb[k][:, 128 * m : 128 * m + 128],
                            alf[k][:],
                            start=(k == 0),
                            stop=(k == 2),
                        )
                    asb = work.tile([128, 512], bf16, tag="aosb", bufs=2, name="asb")
                    nc.scalar.activation(asb[:], pt[:], AF.Copy, bias=0.0)
                    nc.sync.dma_start(out=aot[128 * m : 128 * m + 128, sl], in_=asb[:])

            # schedule: projections upfront (dense full-array work holds the
            # clock gate at K=8); per-qb vl prefetch batches double as promote
            # bursts; each vu is deferred one query block behind its AllGather
            for nb in range(4):
                qk_proj(nb)
            for i in range(4):
                vl_proj(i)
            emit_attention(0)
            for i in range(4, 8):
                vl_proj(i)
            emit_normalize(0)
            emit_attention(1)
            for i in range(8, 12):
                vl_proj(i)
            emit_vu(0)
            emit_normalize(1)
            emit_attention(2)
            for i in range(12, 16):
                vl_proj(i)
            emit_vu(1)
            emit_normalize(2)
            emit_attention(3)
            emit_vu(2)
            emit_normalize(3)

            # ---- heater covers the last AllGather + vu drain --------------------
            heat(40, qlT[0])
            emit_vu(3)
            heat(16, qlT[0])

            # ---- scramble (flat reinterpretation) + transposes ------------------
            wo_sb = []
            for k in range(6):
                wt = const.tile([128, HID], bf16, name=f"wo{k}", tag="wo", bufs=6,
                                padded_shape=[128, HID])
                nc.sync.dma_start(out=wt[:], in_=WO[128 * k : 128 * k + 128, :])
                wo_sb.append(wt)

            aot_v = aot[:, :].rearrange("a b -> (a b)").rearrange(
                "(c d) -> c d", c=1024
            )
            # aotT[j][nbu] : scrambled-feature rows 128j..+128, token cols
            # 512*nbu..+512 -- split per nbu so the out-proj can start on the
            # first half while the second half is still transposing
            aotT = [
                [
                    act.tile([128, 512], bf16, name=f"aotT{j}_{nbu}", tag="aotT",
                             bufs=12)
                    for nbu in range(2)
                ]
                for j in range(6)
            ]
            for u in range(8):
                at = work.tile([128, HID], bf16, tag="at", bufs=3, name="at")
                nc.sync.dma_start(out=at[:], in_=aot_v[128 * u : 128 * u + 128, :])
                for j in range(6):
                    ptag = "sc" if j % 2 == 0 else "av"
                    tp = ps.tile([128, 128], bf16, tag=ptag,
                                 bufs=(2 if ptag == "sc" else 3), name="tpp")
                    nc.tensor.transpose(tp[:], at[:, 128 * j : 128 * j + 128], idn_sb[:])
                    dst = aotT[j][u // 4][:, 128 * (u % 4) : 128 * (u % 4) + 128]
                    if j % 2 == 0:
                        nc.vector.tensor_copy(dst, tp[:])
                    else:
                        nc.scalar.activation(dst, tp[:], AF.Copy, bias=0.0)

            # ---- output projection ---------------------------------------------
            heat(16, qlT[0])
            for nb in range(2):
                for m in range(6):
                    pt = ps.tile([128, 512], f32, tag="sc", bufs=2, name="outp",
                                 padded_shape=[128, 1024])
                    for k in range(6):
                        nc.tensor.matmul(
                            pt[:],
                            wo_sb[k][:, 128 * m : 128 * m + 128],
                            aotT[k][nb][:],
                            start=(k == 0),
                            stop=(k == 5),
                        )
                    osb = work.tile([128, 512], f32, tag="osb", bufs=2, name="osb")
                    nc.scalar.activation(
                        osb[:], pt[:], AF.Identity, bias=bo_sb[:, m : m + 1]
                    )
                    nc.sync.dma_start(
                        out=OUTT[128 * m : 128 * m + 128, 512 * nb : 512 * nb + 512],
                        in_=osb[:],
                    )
            # cover the final output DMA drain
            heat(12, qlT[0])

    nc.finalize()
    _CACHE["nc"] = nc
    return nc


def _host_prep(inputs):
    x = np.asarray(inputs["x"], np.float32)
    Wq = np.asarray(inputs["Wq"], np.float32)
    bq = np.asarray(inputs["bq"], np.float32)
    Wkd = np.asarray(inputs["Wkd"], np.float32)
    bkd = np.asarray(inputs["bkd"], np.float32)
    Wvd = np.asarray(inputs["Wvd"], np.float32)
    bvd = np.asarray(inputs["bvd"], np.float32)
    Wvu = np.asarray(inputs["Wvu"], np.float32)
    bvu = np.asarray(inputs["bvu"], np.float32)
    Wo = np.asarray(inputs["Wo"], np.float32)
    bo = np.asarray(inputs["bo"], np.float32)

    mask = np.tril(np.ones((128, 128), np.float32)).T.astype(ml_dtypes.bfloat16)
    ident = np.eye(128, dtype=ml_dtypes.bfloat16)
    eg = np.zeros((96, 3), np.float32)
    eb = np.zeros((3, 96), np.float32)
    for g in range(3):
        eg[32 * g + 16, g] = 1.0
        eb[g, 32 * g : 32 * g + 17] = 1.0

    bo_mat = np.ascontiguousarray(bo.reshape(6, 128).T)

    per_half = []
    for p in range(2):
        wq_pack = np.zeros((HID, 256), np.float32)
        wk_pack = np.zeros((HID, 256), np.float32)
        wv_pack = np.zeros((HID + 1, 224), np.float32)
        qkb = np.zeros((128, 4), np.float32)
        for hl in range(HL):
            hg = HL * p + hl
            t, g = hl // 3, hl % 3
            cols = slice(128 * t + 32 * g, 128 * t + 32 * g + 16)
            rows = slice(32 * g, 32 * g + 16)
            wq_pack[:, cols] = Wq[:, DH * hg : DH * hg + LD]
            qkb[rows, 0 + t] = bq[DH * hg : DH * hg + LD]
            wk_pack[:, cols] = Wkd[:, LD * hg : LD * hg + LD]
            qkb[rows, 2 + t] = bkd[LD * hg : LD * hg + LD]
            c0 = 32 * hl
            wv_pack[:HID, c0 : c0 + 16] = Wvd[:, LD * hg : LD * hg + LD]
            wv_pack[HID, c0 : c0 + 16] = bvd[LD * hg : LD * hg + LD]
            wv_pack[HID, c0 + 16] = 1.0
        wvu_pack = np.zeros((384, MHALF), np.float32)
        for hp in range(H):
            wvu_pack[32 * hp : 32 * hp + 16, :] = Wvu[
                LD * hp : LD * hp + LD, MHALF * p : MHALF * p + MHALF
            ]
        wvu_pack[16, :] = bvu[MHALF * p : MHALF * p + MHALF]
        per_half.append((
            wq_pack.astype(ml_dtypes.bfloat16),
            wk_pack.astype(ml_dtypes.bfloat16),
            wv_pack.astype(ml_dtypes.bfloat16),
            qkb, wvu_pack,
        ))

    in_maps = []
    for c in range(N_CORES):
        b, p = c // 2, c % 2
        xt = np.concatenate(
            [np.ascontiguousarray(x[b].T), np.ones((1, S), np.float32)], axis=0
        ).astype(ml_dtypes.bfloat16)
        wq_pack, wk_pack, wv_pack, qkb, wvu_pack = per_half[p]
        in_maps.append(
            dict(
                xt=xt, wq=wq_pack, wk=wk_pack, wv=wv_pack, qkb=qkb,
                wvu=wvu_pack,
                wo=Wo.astype(ml_dtypes.bfloat16), bo=bo_mat, mask=mask,
                ident=ident, eg=eg, eb=eb,
            )
        )
    return in_maps


def _run(inputs, **kw):
    nc = _build_nc()
    in_maps = _host_prep(inputs)
    return run_bass_kernel_spmd(nc, in_maps, core_ids=list(range(N_CORES)), **kw)


def kernel(**inputs):
    res = _run(inputs)
    out = np.empty((B, S, HID), np.float32)
    for b in range(B):
        for p in range(2):
            out[b, 1024 * p : 1024 * p + 1024, :] = res.results[2 * b + p]["outt"].T
    return out


# revision 34
# speedup vs baseline: 1.0536x; 1.0018x over previous
"""Trainium2 Bass kernel for nn_CausalSelfAttention_18519898980537.

Low-rank causal self-attention (12 heads, head_dim 64, only the first 16
query dims enter the scores; latent k/v dim 16/head), with the
reference's transpose-reshape scramble before the output projection.

Sharding (8 cores, SPMD single program, per-core differences are input
data only): core c = 2*b + p handles batch b with heads [6p, 6p+6) over
the full causal range.  Per query block, the normalized attention
output alT [2x96, 512] is exchanged pairwise via AllGather so both
cores of a pair see all 12 heads; each core then computes
output-feature half p of ao^T = Wvu^T @ alT, writes it to a DRAM
scratch whose flat reinterpretation is exactly the reference's
reshape(transpose(ao)) for its token half, PE-transposes the re-read
tiles, and applies its token-half of the output projection.

Optimizations vs the original baseline (471us -> ~355us):
- bf16 x/weights for the q/k/v projections (separate overlappable
  LDWEIGHTS instead of f32r self-loads; half the xT DMA), biases folded
  into the scalar-engine projection copies (per-partition bias AP).
- exp windows packed contiguously (diagonal kt pairs shifted left) so
  the scalar engine only exponentiates valid columns.
- fp8-e4m3 DoubleRow attention-value matmuls: off-diagonal kt-tile
  pairs are contracted two-at-a-time ([p,2,c] interleaved operands);
  per-head PSUM accumulator banks at partition 0 (the ISA limits
  DoubleRow destinations to col_grp quadrant 0/64); the softmax
  denominator sums the same quantized probs so fp8 bias cancels.
- heater matmul chains (full-array accumulating bf16) warm the PE_HAM
  clock gate during the initial DMA, the exposed last AllGather, and
  ahead of the output projection; low-utilization attention matmuls can
  never hold K=8/8, so the attention phase targets dense 1.2 GHz issue.
- each query block's value-up projection is deferred one full query
  block behind its AllGather so the PE queue never stalls on the
  collective; per-half normalize right after each attention half.
- bf16 tail datapath (ao scratch, scramble transposes, Wo) and the
  out-proj bias via activation; scramble split per token-half so the
  output projection overlaps the second half's transposes.
"""

import os
import sys

import numpy as np

for _p in ("/opt/trn_rl_repo", "/root/.axon_site/_ro/trn_rl_repo"):
    if os.path.isdir(_p) and _p not in sys.path:
        sys.path.insert(0, _p)

import ml_dtypes  # noqa: E402
import concourse.bacc as bacc  # noqa: E402
import concourse.mybir as mybir  # noqa: E402
from concourse.tile import TileContext  # noqa: E402
from concourse.bass_utils import run_bass_kernel_spmd  # noqa: E402

H, DH, LD, HID, LAT = 12, 64, 16, 768, 192
B, S = 4, 2048
N_CORES = 8
HL = 6  # heads per core
MHALF = HID // 2  # 384 = aoT rows per core
SCALE = LD ** -0.5

f32 = mybir.dt.float32
f32r = mybir.dt.float32r
bf16 = mybir.dt.bfloat16
f8 = mybir.dt.float8e4
AF = mybir.ActivationFunctionType
MULT = mybir.AluOpType.mult

_CACHE = {}


def _build_nc():
    if "nc" in _CACHE:
        return _CACHE["nc"]
    nc = bacc.Bacc("TRN2", target_bir_lowering=False, debug=False, num_devices=N_CORES)

    XT = nc.dram_tensor("xt", [HID + 1, S], bf16, kind="ExternalInput")
    WQ = nc.dram_tensor("wq", [HID, 256], bf16, kind="ExternalInput")
    WK = nc.dram_tensor("wk", [HID, 256], bf16, kind="ExternalInput")
    WV = nc.dram_tensor("wv", [HID + 1, 224], bf16, kind="ExternalInput")
    QKB = nc.dram_tensor("qkb", [128, 4], f32, kind="ExternalInput")
    WVU = nc.dram_tensor("wvu", [384, MHALF], f32r, kind="ExternalInput")
    WO = nc.dram_tensor("wo", [HID, HID], bf16, kind="ExternalInput")
    BO = nc.dram_tensor("bo", [128, 6], f32, kind="ExternalInput")
    MSK = nc.dram_tensor("mask", [128, 128], bf16, kind="ExternalInput")
    IDN = nc.dram_tensor("ident", [128, 128], bf16, kind="ExternalInput")
    EG = nc.dram_tensor("eg", [96, 3], f32r, kind="ExternalInput")
    EB = nc.dram_tensor("eb", [3, 96], f32r, kind="ExternalInput")
    OUTT = nc.dram_tensor("outt", [HID, S // 2], f32, kind="ExternalOutput")

    rg = [[0, 1], [2, 3], [4, 5], [6, 7]]

    with TileContext(nc) as tc:
        with (
            tc.tile_pool(name="const", bufs=1) as const,
            tc.tile_pool(name="act", bufs=1) as act,
            tc.tile_pool(name="work", bufs=1) as work,
            tc.tile_pool(name="ps", bufs=1, space="PSUM") as ps,
            tc.tile_pool(name="dram", bufs=1, space="DRAM") as dram,
        ):
            # ---- small constants first (heater depends on msk) -----------------
            msk_sb = const.tile([128, 128], bf16, name="msk")
            nc.sync.dma_start(out=msk_sb[:], in_=MSK[:, :])
            idn_sb = const.tile([128, 128], bf16, name="idn")
            nc.sync.dma_start(out=idn_sb[:], in_=IDN[:, :])
            eg_sb = const.tile([96, 3], f32r, name="eg")
            nc.sync.dma_start(out=eg_sb[:], in_=EG[:, :])
            eb_sb = const.tile([3, 96], f32r, name="eb")
            nc.sync.dma_start(out=eb_sb[:], in_=EB[:, :])
            bo_sb = const.tile([128, 6], f32, name="bo")
            nc.sync.dma_start(out=bo_sb[:], in_=BO[:, :])

            hsrc = const.tile([128, 512], bf16, name="hsrc")
            for r in range(4):
                nc.sync.dma_start(out=hsrc[:, 128 * r : 128 * r + 128], in_=MSK[:, :])

            # per-burst accumulation chain: no PSUM group resets inside a
            # burst, so heater matmuls run gap-free and the PE_HAM clock gate
            # sees a fully-busy array (promoting to K=8/8, 2.4 GHz)
            def heat(n, rhs=None):
                src = hsrc if rhs is None else rhs
                hp = ps.tile([128, 512], f32, tag="pg", bufs=1, name="heatp")
                for i in range(n):
                    nc.tensor.matmul(
                        hp[:], msk_sb[:, 0:128], src[:, 0:512],
                        start=(i == 0), stop=(i == n - 1),
                        skip_group_check=True,
                    )

            # warm the clock gate while the big xT DMA lands
            heat(52)

            # ---- weight + xT loads ---------------------------------------------
            def load_w(handle, ncols, nm, nk):
                tiles = []
                for k in range(nk):
                    pp = 128 if k < 6 else 1
                    t = const.tile(
                        [pp, ncols], bf16, name=f"{nm}{k}", tag=nm, bufs=nk,
                        padded_shape=[128, ncols],
                    )
                    nc.sync.dma_start(out=t[:], in_=handle[128 * k : 128 * k + pp, :])
                    tiles.append(t)
                return tiles

            xt_sb = []
            for k in range(7):
                pp = 128 if k < 6 else 1
                t = const.tile([pp, S], bf16, name=f"xt{k}")
                for hh in range(2):
                    nc.sync.dma_start(
                        out=t[:, 1024 * hh : 1024 * hh + 1024],
                        in_=XT[128 * k : 128 * k + pp, 1024 * hh : 1024 * hh + 1024],
                    )
                xt_sb.append(t)

            wq_sb = load_w(WQ, 256, "wq", 6)
            wk_sb = load_w(WK, 256, "wk", 6)
            wv_sb = load_w(WV, 224, "wv", 7)
            qkb_sb = const.tile([128, 4], f32, name="qkb")
            nc.sync.dma_start(out=qkb_sb[:], in_=QKB[:, :])

            # ---- q/k projections (bf16 outputs feed the score matmuls) ---------
            qlT = [act.tile([128, S], bf16, name=f"qlT{t}") for t in range(2)]
            klT = [act.tile([128, S], bf16, name=f"klT{t}") for t in range(2)]

            def qk_proj(nb):
                # 512-token block of q and k projections; bf16 stationaries
                # use overlappable LDWEIGHTS (f32r self-loads serially); the
                # per-head bias is folded into the activation copy
                for wi, (wt, dst) in enumerate(((wq_sb, qlT), (wk_sb, klT))):
                    for t in range(2):
                        ptag = "sc" if (wi + t) % 2 == 0 else "av"
                        pt = ps.tile([128, 512], f32, tag=ptag,
                                     bufs=(2 if ptag == "sc" else 3),
                                     name="projp")
                        for k in range(6):
                            nc.tensor.matmul(
                                pt[:],
                                wt[k][:, 128 * t : 128 * t + 128],
                                xt_sb[k][:, 512 * nb : 512 * nb + 512],
                                start=(k == 0),
                                stop=(k == 5),
                            )
                        nc.scalar.activation(
                            dst[t][:, 512 * nb : 512 * nb + 512], pt[:],
                            AF.Identity, bias=qkb_sb[:, 2 * wi + t : 2 * wi + t + 1],
                        )

            vl_sb = [act.tile([128, 192], bf16, name=f"vl{i}") for i in range(16)]
            # fp8 copies packed in DoubleRow pair layout: vlp[j][:, 0:192] is
            # kt=2j, [192:384] is kt=2j+1 -> lhsT [128, 2, 32-band] slices
            vlp = [act.tile([128, 384], f8, name=f"vlp{j}") for j in range(8)]

            def vl_proj(i):
                ptag = "sc" if i % 2 == 0 else "av"
                pt = ps.tile([128, 224], f32, tag=ptag,
                             bufs=(2 if ptag == "sc" else 3), name="projv")
                for k in range(7):
                    nc.tensor.matmul(
                        pt[:],
                        xt_sb[k][:, 128 * i : 128 * i + 128],
                        wv_sb[k][:],
                        start=(k == 0),
                        stop=(k == 6),
                    )
                nc.vector.tensor_copy(vl_sb[i][:], pt[:, 0:192])
                with nc.allow_low_precision(reason="fp8 DoubleRow attention AV"):
                    nc.vector.tensor_copy(
                        vlp[i // 2][:, 192 * (i % 2) : 192 * (i % 2) + 192],
                        pt[:, 0:192],
                    )

            # ---- attention (software-pipelined over kt pairs) -------------------
            alt_t = [act.tile([96, S], f32r, name=f"alt{t}") for t in range(2)]
            cin = [dram.tile([192, 512], f32r, name=f"cin{c}") for c in range(4)]
            cout = [dram.tile([384, 512], f32r, name=f"cout{c}") for c in range(4)]
            wvu_sb = [const.tile([128, MHALF], f32r, name=f"wvu{k}") for k in range(3)]
            for k in range(3):
                nc.sync.dma_start(out=wvu_sb[k][:], in_=WVU[128 * k : 128 * k + 128, :])
            aot = dram.tile([MHALF, S], bf16, name="aot")

            def emit_attention(qb):
                for t in range(2):
                    # one accumulator bank per head, all at partition 0:
                    # DoubleRow destinations are only legal in the first PSUM
                    # quadrant group, and 32-offset bands are rejected
                    avps = [
                        ps.tile([128, 512], f32, tag="av", bufs=3, name="avp")
                        for _ in range(3)
                    ]
                    nkt = 4 * qb + 4
                    npair = nkt // 2
                    exs = {}

                    def win(j, qb=qb):
                        # packed windows: ii=1 shifted left so exp'd cols
                        # are contiguous [0, w0+w1)
                        kts = (2 * j, 2 * j + 1)
                        offs = [max(0, 128 * (kt - 4 * qb)) for kt in kts]
                        ws = [512 - o for o in offs]
                        bases = [0, ws[0]]
                        return kts, offs, ws, bases

                    def emit_scores(j, t=t, qb=qb, exs=exs):
                        kts, offs, ws, bases = win(j)
                        W = ws[0] + ws[1]
                        for g in range(3):
                            scp = ps.tile([128, 1024], f32, tag="sc", bufs=2,
                                          name="scp")
                            for ii, kt in enumerate(kts):
                                nc.tensor.matmul(
                                    scp[:, bases[ii] : bases[ii] + ws[ii]],
                                    klT[t][
                                        32 * g : 32 * g + 16,
                                        128 * kt : 128 * kt + 128,
                                    ],
                                    qlT[t][
                                        32 * g : 32 * g + 16,
                                        512 * qb + offs[ii] : 512 * qb + 512,
                                    ],
                                    start=True,
                                    stop=True,
                                    tile_position=(32 * g, 0),
                                )
                            diag = kts[1] - 4 * qb >= 0
                            ex = work.tile([128, 1024], bf16 if diag else f8,
                                           tag="expT", bufs=6, name="ex")
                            with nc.allow_low_precision(
                                reason="fp8 probs; denominator sums the same"
                                " quantized values so the ratio cancels"
                            ):
                                nc.scalar.activation(
                                    ex[:, 0:W], scp[:, 0:W], AF.Exp, bias=0.0,
                                    scale=SCALE,
                                )
                            for ii, kt in enumerate(kts):
                                if kt - 4 * qb >= 0:
                                    nc.vector.tensor_tensor(
                                        ex[:, bases[ii] : bases[ii] + 128],
                                        ex[:, bases[ii] : bases[ii] + 128],
                                        msk_sb[:],
                                        op=MULT,
                                    )
                            exs[(j, g)] = ex

                    def emit_av(j, t=t, qb=qb, nkt=nkt, avps=avps, exs=exs):
                        kts, offs, ws, bases = win(j)
                        diag = kts[1] - 4 * qb >= 0
                        for g in range(3):
                            hh = 3 * t + g
                            dst, pos = avps[g], 0
                            ex = exs.pop((j, g))
                            if not diag:
                                # both kt tiles in one fp8 DoubleRow matmul
                                nc.tensor.matmul(
                                    dst[pos : pos + 32, 0:512],
                                    vlp[j][:].rearrange(
                                        "p (two c) -> p two c", two=2
                                    )[:, :, 32 * hh : 32 * hh + 32],
                                    ex[:].rearrange(
                                        "p (two c) -> p two c", two=2
                                    ),
                                    start=(kts[0] == 0),
                                    stop=(kts[1] == nkt - 1),
                                    tile_position=(0, pos),
                                    skip_group_check=True,
                                    perf_mode=mybir.MatmulPerfMode.DoubleRow,
                                )
                                continue
                            for ii, kt in enumerate(kts):
                                nc.tensor.matmul(
                                    dst[
                                        pos : pos + 32,
                                        offs[ii] : offs[ii] + ws[ii],
                                    ],
                                    vl_sb[kt][:, 32 * hh : 32 * hh + 32],
                                    ex[:, bases[ii] : bases[ii] + ws[ii]],
                                    start=(kt == 0),
                                    stop=(kt == nkt - 1),
                                    tile_position=(0, pos),
                                    skip_group_check=True,
                                )
                            if kts[1] == nkt - 1:
                                # this head is done: drain its accumulator
                                # while the other heads' matmuls continue
                                nc.vector.tensor_copy(
                                    alt_t[t][
                                        32 * g : 32 * g + 32,
                                        512 * qb : 512 * qb + 512,
                                    ],
                                    dst[0:32, :],
                                )

                    emit_scores(0)
                    for j in range(1, npair):
                        emit_scores(j)
                        if j == 1:
                            # the previous half's normalize goes here: its smp
                            # matmul then reaches the PE queue head well after
                            # the DVE accumulator drains it depends on
                            while pending_norm:
                                normalize_half(*pending_norm.pop(0))
                        emit_av(j - 1)
                    emit_av(npair - 1)

                    pending_norm.append((qb, t))

            def normalize_half(qb, tt):
                # normalize + stage this half for exchange right after its
                # attention half finishes (shortens the last pre-AG chain)
                sl = slice(512 * qb, 512 * qb + 512)
                smp = ps.tile([3, 512], f32, tag="pg", bufs=1, name="smp")
                nc.tensor.matmul(
                    smp[:], eg_sb[:], alt_t[tt][:, sl], start=True, stop=True
                )
                rcp = work.tile([3, 512], f32r, tag="recip", bufs=2, name="rcp")
                with nc.allow_low_precision(reason="recip feeds fp32r matmul"):
                    nc.vector.reciprocal(rcp[:], smp[:])
                bcp = ps.tile([96, 512], f32, tag="pg", bufs=1, name="bcp")
                nc.tensor.matmul(bcp[:], eb_sb[:], rcp[:], start=True, stop=True)
                nc.vector.tensor_tensor(
                    alt_t[tt][:, sl], alt_t[tt][:, sl], bcp[:], op=MULT
                )
                nc.sync.dma_start(
                    out=cin[qb][96 * tt : 96 * tt + 96, :], in_=alt_t[tt][:, sl]
                )

            pending_norm = []

            def emit_normalize(qb):
                while pending_norm:
                    normalize_half(*pending_norm.pop(0))
                nc.gpsimd.collective_compute(
                    "AllGather",
                    mybir.AluOpType.bypass,
                    replica_groups=rg,
                    ins=[cin[qb].opt()],
                    outs=[cout[qb].opt()],
                )

            def emit_vu(qb):
                # deferred one full query block behind the AllGather that
                # feeds it, so the PE queue never stalls on the collective
                sl = slice(512 * qb, 512 * qb + 512)
                alf = [
                    act.tile([128, 512], f32r, tag="alf", bufs=6, name=f"alf{k}")
                    for k in range(3)
                ]
                for k in range(3):
                    nc.sync.dma_start(
                        out=alf[k][:], in_=cout[qb][128 * k : 128 * k + 128, :]
                    )
                for m in range(3):
                    pt = ps.tile([128, 512], f32, tag="sc", bufs=2, name="vup",
                                 padded_shape=[128, 1024])
                    for k in range(3):
                        nc.tensor.matmul(
                            pt[:],
                            wvu_sb[k][:, 128 * m : 128 * m + 128],
                            alf[k][:],
                            start=(k == 0),
                            stop=(k == 2),
                        )
                    asb = work.tile([128, 512], bf16, tag="aosb", bufs=2, name="asb")
                    nc.scalar.activation(asb[:], pt[:], AF.Copy, bias=0.0)
                    nc.sync.dma_start(out=aot[128 * m : 128 * m + 128, sl], in_=asb[:])

            # schedule: projections upfront (dense full-array work holds the
            # clock gate at K=8); per-qb vl prefetch batches double as promote
            # bursts; each vu is deferred one query block behind its AllGather
            for nb in range(4):
                qk_proj(nb)
            for i in range(4):
                vl_proj(i)
            emit_attention(0)
            for i in range(4, 8):
                vl_proj(i)
            emit_normalize(0)
            emit_attention(1)
            for i in range(8, 12):
                vl_proj(i)
            emit_vu(0)
            emit_normalize(1)
            emit_attention(2)
            for i in range(12, 16):
                vl_proj(i)
            emit_vu(1)
            emit_normalize(2)
            emit_attention(3)
            emit_vu(2)
            emit_normalize(3)

            # ---- heater covers the last AllGather + vu drain --------------------
            heat(40, qlT[0])
            emit_vu(3)
            heat(16, qlT[0])

            # ---- scramble (flat reinterpretation) + transposes ------------------
            wo_sb = []
            for k in range(6):
                wt = const.tile([128, HID], bf16, name=f"wo{k}", tag="wo", bufs=6,
                                padded_shape=[128, HID])
                nc.sync.dma_start(out=wt[:], in_=WO[128 * k : 128 * k + 128, :])
                wo_sb.append(wt)

            aot_v = aot[:, :].rearrange("a b -> (a b)").rearrange(
                "(c d) -> c d", c=1024
            )
            # aotT[j][nbu] : scrambled-feature rows 128j..+128, token cols
            # 512*nbu..+512 -- split per nbu so the out-proj can start on the
            # first half while the second half is still transposing
            aotT = [
                [
                    act.tile([128, 512], bf16, name=f"aotT{j}_{nbu}", tag="aotT",
                             bufs=12)
                    for nbu in range(2)
                ]
                for j in range(6)
            ]
            for u in range(8):
                at = work.tile([128, HID], bf16, tag="at", bufs=3, name="at")
                nc.sync.dma_start(out=at[:], in_=aot_v[128 * u : 128 * u + 128, :])
                for j in range(6):
                    ptag = "sc" if j % 2 == 0 else "av"
                    tp = ps.tile([128, 128], bf16, tag=ptag,
                                 bufs=(2 if ptag == "sc" else 3), name="tpp")
                    nc.tensor.transpose(tp[:], at[:, 128 * j : 128 * j + 128], idn_sb[:])
                    dst = aotT[j][u // 4][:, 128 * (u % 4) : 128 * (u % 4) + 128]
                    if j % 2 == 0:
                        nc.vector.tensor_copy(dst, tp[:])
                    else:
                        nc.scalar.activation(dst, tp[:], AF.Copy, bias=0.0)

            # ---- output projection ---------------------------------------------
            heat(16, qlT[0])
            for nb in range(2):
                for m in range(6):
                    pt = ps.tile([128, 512], f32, tag="sc", bufs=2, name="outp",
                                 padded_shape=[128, 1024])
                    for k in range(6):
                        nc.tensor.matmul(
                            pt[:],
                            wo_sb[k][:, 128 * m : 128 * m + 128],
                            aotT[k][nb][:],
                            start=(k == 0),
                            stop=(k == 5),
                        )
                    osb = work.tile([128, 512], f32, tag="osb", bufs=2, name="osb")
                    nc.scalar.activation(
                        osb[:], pt[:], AF.Identity, bias=bo_sb[:, m : m + 1]
                    )
                    nc.sync.dma_start(
                        out=OUTT[128 * m : 128 * m + 128, 512 * nb : 512 * nb + 512],
                        in_=osb[:],
                    )
            # cover the final output DMA drain
            heat(12, qlT[0])

    nc.finalize()
    _CACHE["nc"] = nc
    return nc


def _host_prep(inputs):
    x = np.asarray(inputs["x"], np.float32)
    Wq = np.asarray(inputs["Wq"], np.float32)
    bq = np.asarray(inputs["bq"], np.float32)
    Wkd = np.asarray(inputs["Wkd"], np.float32)
    bkd = np.asarray(inputs["bkd"], np.float32)
    Wvd = np.asarray(inputs["Wvd"], np.float32)
    bvd = np.asarray(inputs["bvd"], np.float32)
    Wvu = np.asarray(inputs["Wvu"], np.float32)
    bvu = np.asarray(inputs["bvu"], np.float32)
    Wo = np.asarray(inputs["Wo"], np.float32)
    bo = np.asarray(inputs["bo"], np.float32)

    mask = np.tril(np.ones((128, 128), np.float32)).T.astype(ml_dtypes.bfloat16)
    ident = np.eye(128, dtype=ml_dtypes.bfloat16)
    eg = np.zeros((96, 3), np.float32)
    eb = np.zeros((3, 96), np.float32)
    for g in range(3):
        eg[32 * g + 16, g] = 1.0
        eb[g, 32 * g : 32 * g + 17] = 1.0

    bo_mat = np.ascontiguousarray(bo.reshape(6, 128).T)

    per_half = []
    for p in range(2):
        wq_pack = np.zeros((HID, 256), np.float32)
        wk_pack = np.zeros((HID, 256), np.float32)
        wv_pack = np.zeros((HID + 1, 224), np.float32)
        qkb = np.zeros((128, 4), np.float32)
        for hl in range(HL):
            hg = HL * p + hl
            t, g = hl // 3, hl % 3
            cols = slice(128 * t + 32 * g, 128 * t + 32 * g + 16)
            rows = slice(32 * g, 32 * g + 16)
            wq_pack[:, cols] = Wq[:, DH * hg : DH * hg + LD]
            qkb[rows, 0 + t] = bq[DH * hg : DH * hg + LD]
            wk_pack[:, cols] = Wkd[:, LD * hg : LD * hg + LD]
            qkb[rows, 2 + t] = bkd[LD * hg : LD * hg + LD]
            c0 = 32 * hl
            wv_pack[:HID, c0 : c0 + 16] = Wvd[:, LD * hg : LD * hg + LD]
            wv_pack[HID, c0 : c0 + 16] = bvd[LD * hg : LD * hg + LD]
            wv_pack[HID, c0 + 16] = 1.0
        wvu_pack = np.zeros((384, MHALF), np.float32)
        for hp in range(H):
            wvu_pack[32 * hp : 32 * hp + 16, :] = Wvu[
                LD * hp : LD * hp + LD, MHALF * p : MHALF * p + MHALF
            ]
        wvu_pack[16, :] = bvu[MHALF * p : MHALF * p + MHALF]
        per_half.append((
            wq_pack.astype(ml_dtypes.bfloat16),
            wk_pack.astype(ml_dtypes.bfloat16),
            wv_pack.astype(ml_dtypes.bfloat16),
            qkb, wvu_pack,
        ))

    in_maps = []
    for c in range(N_CORES):
        b, p = c // 2, c % 2
        xt = np.concatenate(
            [np.ascontiguousarray(x[b].T), np.ones((1, S), np.float32)], axis=0
        ).astype(ml_dtypes.bfloat16)
        wq_pack, wk_pack, wv_pack, qkb, wvu_pack = per_half[p]
        in_maps.append(
            dict(
                xt=xt, wq=wq_pack, wk=wk_pack, wv=wv_pack, qkb=qkb,
                wvu=wvu_pack,
                wo=Wo.astype(ml_dtypes.bfloat16), bo=bo_mat, mask=mask,
                ident=ident, eg=eg, eb=eb,
            )
        )
    return in_maps


def _run(inputs, **kw):
    nc = _build_nc()
    in_maps = _host_prep(inputs)
    return run_bass_kernel_spmd(nc, in_maps, core_ids=list(range(N_CORES)), **kw)


def kernel(**inputs):
    res = _run(inputs)
    out = np.empty((B, S, HID), np.float32)
    for b in range(B):
        for p in range(2):
            out[b, 1024 * p : 1024 * p + 1024, :] = res.results[2 * b + p]["outt"].T
    return out


# revision 36
# speedup vs baseline: 1.0620x; 1.0080x over previous
"""Trainium2 Bass kernel for nn_CausalSelfAttention_18519898980537.

Low-rank causal self-attention (12 heads, head_dim 64, only the first 16
query dims enter the scores; latent k/v dim 16/head), with the
reference's transpose-reshape scramble before the output projection.

Sharding (8 cores, SPMD single program, per-core differences are input
data only): core c = 2*b + p handles batch b with heads [6p, 6p+6) over
the full causal range.  Per query block, the normalized attention
output alT [2x96, 512] is exchanged pairwise via AllGather so both
cores of a pair see all 12 heads; each core then computes
output-feature half p of ao^T = Wvu^T @ alT, writes it to a DRAM
scratch whose flat reinterpretation is exactly the reference's
reshape(transpose(ao)) for its token half, PE-transposes the re-read
tiles, and applies its token-half of the output projection.

Optimizations vs the original baseline (471us -> ~344us):
- bf16 x/weights for the q/k/v projections (separate overlappable
  LDWEIGHTS instead of f32r self-loads; half the xT DMA), biases folded
  into the scalar-engine projection copies (per-partition bias AP).
- exp windows packed contiguously (diagonal kt pairs shifted left) so
  the scalar engine only exponentiates valid columns.
- fp8-e4m3 DoubleRow attention-value matmuls: off-diagonal kt-tile
  pairs are contracted two-at-a-time ([p,2,c] interleaved operands);
  per-head PSUM accumulator banks at partition 0 (the ISA limits
  DoubleRow destinations to col_grp quadrant 0/64); the softmax
  denominator sums the same quantized probs so fp8 bias cancels.
- heater matmul chains (full-array accumulating bf16) warm the PE_HAM
  clock gate during the initial DMA, the exposed last AllGather, and
  ahead of the output projection; low-utilization attention matmuls can
  never hold K=8/8, so the attention phase targets dense 1.2 GHz issue.
- each query block's value-up projection is deferred one full query
  block behind its AllGather so the PE queue never stalls on the
  collective; each half's normalize is deferred into the next half's
  attention stream so its smp matmul never blocks on the DVE drain.
- bf16 tail datapath (ao scratch, scramble transposes, Wo) and the
  out-proj bias via activation; scramble split per token-half so the
  output projection overlaps the second half's transposes.
"""

import os
import sys

import numpy as np

for _p in ("/opt/trn_rl_repo", "/root/.axon_site/_ro/trn_rl_repo"):
    if os.path.isdir(_p) and _p not in sys.path:
        sys.path.insert(0, _p)

import ml_dtypes  # noqa: E402
import concourse.bacc as bacc  # noqa: E402
import concourse.mybir as mybir  # noqa: E402
from concourse.tile import TileContext  # noqa: E402
from concourse.bass_utils import run_bass_kernel_spmd  # noqa: E402

H, DH, LD, HID, LAT = 12, 64, 16, 768, 192
B, S = 4, 2048
N_CORES = 8
HL = 6  # heads per core
MHALF = HID // 2  # 384 = aoT rows per core
SCALE = LD ** -0.5

f32 = mybir.dt.float32
f32r = mybir.dt.float32r
bf16 = mybir.dt.bfloat16
f8 = mybir.dt.float8e4
AF = mybir.ActivationFunctionType
MULT = mybir.AluOpType.mult

_CACHE = {}


def _build_nc():
    if "nc" in _CACHE:
        return _CACHE["nc"]
    nc = bacc.Bacc("TRN2", target_bir_lowering=False, debug=False, num_devices=N_CORES)

    XT = nc.dram_tensor("xt", [HID + 1, S], bf16, kind="ExternalInput")
    WQ = nc.dram_tensor("wq", [HID, 256], bf16, kind="ExternalInput")
    WK = nc.dram_tensor("wk", [HID, 256], bf16, kind="ExternalInput")
    WV = nc.dram_tensor("wv", [HID + 1, 224], bf16, kind="ExternalInput")
    QKB = nc.dram_tensor("qkb", [128, 4], f32, kind="ExternalInput")
    WVU = nc.dram_tensor("wvu", [384, MHALF], f32r, kind="ExternalInput")
    WO = nc.dram_tensor("wo", [HID, HID], bf16, kind="ExternalInput")
    BO = nc.dram_tensor("bo", [128, 6], f32, kind="ExternalInput")
    MSK = nc.dram_tensor("mask", [128, 128], bf16, kind="ExternalInput")
    IDN = nc.dram_tensor("ident", [128, 128], bf16, kind="ExternalInput")
    EG = nc.dram_tensor("eg", [96, 3], f32r, kind="ExternalInput")
    EB = nc.dram_tensor("eb", [3, 96], f32r, kind="ExternalInput")
    OUTT = nc.dram_tensor("outt", [HID, S // 2], f32, kind="ExternalOutput")

    rg = [[0, 1], [2, 3], [4, 5], [6, 7]]

    with TileContext(nc) as tc:
        with (
            tc.tile_pool(name="const", bufs=1) as const,
            tc.tile_pool(name="act", bufs=1) as act,
            tc.tile_pool(name="work", bufs=1) as work,
            tc.tile_pool(name="ps", bufs=1, space="PSUM") as ps,
            tc.tile_pool(name="dram", bufs=1, space="DRAM") as dram,
        ):
            # ---- small constants first (heater depends on msk) -----------------
            msk_sb = const.tile([128, 128], bf16, name="msk")
            nc.sync.dma_start(out=msk_sb[:], in_=MSK[:, :])
            idn_sb = const.tile([128, 128], bf16, name="idn")
            nc.sync.dma_start(out=idn_sb[:], in_=IDN[:, :])
            eg_sb = const.tile([96, 3], f32r, name="eg")
            nc.sync.dma_start(out=eg_sb[:], in_=EG[:, :])
            eb_sb = const.tile([3, 96], f32r, name="eb")
            nc.sync.dma_start(out=eb_sb[:], in_=EB[:, :])
            bo_sb = const.tile([128, 6], f32, name="bo")
            nc.sync.dma_start(out=bo_sb[:], in_=BO[:, :])

            hsrc = const.tile([128, 512], bf16, name="hsrc")
            for r in range(4):
                nc.sync.dma_start(out=hsrc[:, 128 * r : 128 * r + 128], in_=MSK[:, :])

            # per-burst accumulation chain: no PSUM group resets inside a
            # burst, so heater matmuls run gap-free and the PE_HAM clock gate
            # sees a fully-busy array (promoting to K=8/8, 2.4 GHz)
            def heat(n, rhs=None):
                src = hsrc if rhs is None else rhs
                hp = ps.tile([128, 512], f32, tag="pg", bufs=1, name="heatp")
                for i in range(n):
                    nc.tensor.matmul(
                        hp[:], msk_sb[:, 0:128], src[:, 0:512],
                        start=(i == 0), stop=(i == n - 1),
                        skip_group_check=True,
                    )

            # warm the clock gate while the big xT DMA lands
            heat(52)

            # ---- weight + xT loads ---------------------------------------------
            def load_w(handle, ncols, nm, nk):
                tiles = []
                for k in range(nk):
                    pp = 128 if k < 6 else 1
                    t = const.tile(
                        [pp, ncols], bf16, name=f"{nm}{k}", tag=nm, bufs=nk,
                        padded_shape=[128, ncols],
                    )
                    nc.sync.dma_start(out=t[:], in_=handle[128 * k : 128 * k + pp, :])
                    tiles.append(t)
                return tiles

            xt_sb = []
            for k in range(7):
                pp = 128 if k < 6 else 1
                t = const.tile([pp, S], bf16, name=f"xt{k}")
                for hh in range(2):
                    nc.sync.dma_start(
                        out=t[:, 1024 * hh : 1024 * hh + 1024],
                        in_=XT[128 * k : 128 * k + pp, 1024 * hh : 1024 * hh + 1024],
                    )
                xt_sb.append(t)

            wq_sb = load_w(WQ, 256, "wq", 6)
            wk_sb = load_w(WK, 256, "wk", 6)
            wv_sb = load_w(WV, 224, "wv", 7)
            qkb_sb = const.tile([128, 4], f32, name="qkb")
            nc.sync.dma_start(out=qkb_sb[:], in_=QKB[:, :])

            # ---- q/k projections (bf16 outputs feed the score matmuls) ---------
            qlT = [act.tile([128, S], bf16, name=f"qlT{t}") for t in range(2)]
            klT = [act.tile([128, S], bf16, name=f"klT{t}") for t in range(2)]

            def qk_proj(nb):
                # 512-token block of q and k projections; bf16 stationaries
                # use overlappable LDWEIGHTS (f32r self-loads serially); the
                # per-head bias is folded into the activation copy
                for wi, (wt, dst) in enumerate(((wq_sb, qlT), (wk_sb, klT))):
                    for t in range(2):
                        ptag = "sc" if (wi + t) % 2 == 0 else "av"
                        pt = ps.tile([128, 512], f32, tag=ptag,
                                     bufs=(2 if ptag == "sc" else 3),
                                     name="projp")
                        for k in range(6):
                            nc.tensor.matmul(
                                pt[:],
                                wt[k][:, 128 * t : 128 * t + 128],
                                xt_sb[k][:, 512 * nb : 512 * nb + 512],
                                start=(k == 0),
                                stop=(k == 5),
                            )
                        nc.scalar.activation(
                            dst[t][:, 512 * nb : 512 * nb + 512], pt[:],
                            AF.Identity, bias=qkb_sb[:, 2 * wi + t : 2 * wi + t + 1],
                        )

            vl_sb = [act.tile([128, 192], bf16, name=f"vl{i}") for i in range(16)]
            # fp8 copies packed in DoubleRow pair layout: vlp[j][:, 0:192] is
            # kt=2j, [192:384] is kt=2j+1 -> lhsT [128, 2, 32-band] slices
            vlp = [act.tile([128, 384], f8, name=f"vlp{j}") for j in range(8)]

            def vl_proj(i):
                ptag = "sc" if i % 2 == 0 else "av"
                pt = ps.tile([128, 224], f32, tag=ptag,
                             bufs=(2 if ptag == "sc" else 3), name="projv")
                for k in range(7):
                    nc.tensor.matmul(
                        pt[:],
                        xt_sb[k][:, 128 * i : 128 * i + 128],
                        wv_sb[k][:],
                        start=(k == 0),
                        stop=(k == 6),
                    )
                nc.vector.tensor_copy(vl_sb[i][:], pt[:, 0:192])
                with nc.allow_low_precision(reason="fp8 DoubleRow attention AV"):
                    nc.vector.tensor_copy(
                        vlp[i // 2][:, 192 * (i % 2) : 192 * (i % 2) + 192],
                        pt[:, 0:192],
                    )

            # ---- attention (software-pipelined over kt pairs) -------------------
            alt_t = [act.tile([96, S], f32r, name=f"alt{t}") for t in range(2)]
            cin = [dram.tile([192, 512], f32r, name=f"cin{c}") for c in range(4)]
            cout = [dram.tile([384, 512], f32r, name=f"cout{c}") for c in range(4)]
            wvu_sb = [const.tile([128, MHALF], f32r, name=f"wvu{k}") for k in range(3)]
            for k in range(3):
                nc.sync.dma_start(out=wvu_sb[k][:], in_=WVU[128 * k : 128 * k + 128, :])
            aot = dram.tile([MHALF, S], bf16, name="aot")

            def emit_attention(qb):
                for t in range(2):
                    # one accumulator bank per head, all at partition 0:
                    # DoubleRow destinations are only legal in the first PSUM
                    # quadrant group, and 32-offset bands are rejected
                    avps = [
                        ps.tile([128, 512], f32, tag="av", bufs=3, name="avp")
                        for _ in range(3)
                    ]
                    nkt = 4 * qb + 4
                    npair = nkt // 2
                    exs = {}

                    def win(j, qb=qb):
                        # packed windows: ii=1 shifted left so exp'd cols
                        # are contiguous [0, w0+w1)
                        kts = (2 * j, 2 * j + 1)
                        offs = [max(0, 128 * (kt - 4 * qb)) for kt in kts]
                        ws = [512 - o for o in offs]
                        bases = [0, ws[0]]
                        return kts, offs, ws, bases

                    def emit_scores(j, t=t, qb=qb, exs=exs):
                        kts, offs, ws, bases = win(j)
                        W = ws[0] + ws[1]
                        for g in range(3):
                            scp = ps.tile([128, 1024], f32, tag="sc", bufs=2,
                                          name="scp")
                            for ii, kt in enumerate(kts):
                                nc.tensor.matmul(
                                    scp[:, bases[ii] : bases[ii] + ws[ii]],
                                    klT[t][
                                        32 * g : 32 * g + 16,
                                        128 * kt : 128 * kt + 128,
                                    ],
                                    qlT[t][
                                        32 * g : 32 * g + 16,
                                        512 * qb + offs[ii] : 512 * qb + 512,
                                    ],
                                    start=True,
                                    stop=True,
                                    tile_position=(32 * g, 0),
                                )
                            diag = kts[1] - 4 * qb >= 0
                            ex = work.tile([128, 1024], bf16 if diag else f8,
                                           tag="expT", bufs=6, name="ex")
                            with nc.allow_low_precision(
                                reason="fp8 probs; denominator sums the same"
                                " quantized values so the ratio cancels"
                            ):
                                nc.scalar.activation(
                                    ex[:, 0:W], scp[:, 0:W], AF.Exp, bias=0.0,
                                    scale=SCALE,
                                )
                            for ii, kt in enumerate(kts):
                                if kt - 4 * qb >= 0:
                                    nc.vector.tensor_tensor(
                                        ex[:, bases[ii] : bases[ii] + 128],
                                        ex[:, bases[ii] : bases[ii] + 128],
                                        msk_sb[:],
                                        op=MULT,
                                    )
                            exs[(j, g)] = ex

                    def emit_av(j, t=t, qb=qb, nkt=nkt, avps=avps, exs=exs):
                        kts, offs, ws, bases = win(j)
                        diag = kts[1] - 4 * qb >= 0
                        for g in range(3):
                            hh = 3 * t + g
                            dst, pos = avps[g], 0
                            ex = exs.pop((j, g))
                            if not diag:
                                # both kt tiles in one fp8 DoubleRow matmul
                                nc.tensor.matmul(
                                    dst[pos : pos + 32, 0:512],
                                    vlp[j][:].rearrange(
                                        "p (two c) -> p two c", two=2
                                    )[:, :, 32 * hh : 32 * hh + 32],
                                    ex[:].rearrange(
                                        "p (two c) -> p two c", two=2
                                    ),
                                    start=(kts[0] == 0),
                                    stop=(kts[1] == nkt - 1),
                                    tile_position=(0, pos),
                                    skip_group_check=True,
                                    perf_mode=mybir.MatmulPerfMode.DoubleRow,
                                )
                                continue
                            for ii, kt in enumerate(kts):
                                nc.tensor.matmul(
                                    dst[
                                        pos : pos + 32,
                                        offs[ii] : offs[ii] + ws[ii],
                                    ],
                                    vl_sb[kt][:, 32 * hh : 32 * hh + 32],
                                    ex[:, bases[ii] : bases[ii] + ws[ii]],
                                    start=(kt == 0),
                                    stop=(kt == nkt - 1),
                                    tile_position=(0, pos),
                                    skip_group_check=True,
                                )
                            if kts[1] == nkt - 1:
                                # this head is done: drain its accumulator
                                # while the other heads' matmuls continue
                                nc.vector.tensor_copy(
                                    alt_t[t][
                                        32 * g : 32 * g + 32,
                                        512 * qb : 512 * qb + 512,
                                    ],
                                    dst[0:32, :],
                                )

                    emit_scores(0)
                    for j in range(1, npair):
                        emit_scores(j)
                        if j == min(2, npair - 1):
                            # the previous half's normalize goes here: its smp
                            # matmul then reaches the PE queue head well after
                            # the DVE accumulator drains it depends on
                            while pending_norm:
                                normalize_half(*pending_norm.pop(0))
                        emit_av(j - 1)
                    emit_av(npair - 1)

                    pending_norm.append((qb, t))

            def normalize_half(qb, tt):
                # normalize + stage this half for exchange right after its
                # attention half finishes (shortens the last pre-AG chain)
                sl = slice(512 * qb, 512 * qb + 512)
                smp = ps.tile([3, 512], f32, tag="pg", bufs=1, name="smp")
                nc.tensor.matmul(
                    smp[:], eg_sb[:], alt_t[tt][:, sl], start=True, stop=True
                )
                rcp = work.tile([3, 512], f32r, tag="recip", bufs=2, name="rcp")
                with nc.allow_low_precision(reason="recip feeds fp32r matmul"):
                    nc.vector.reciprocal(rcp[:], smp[:])
                bcp = ps.tile([96, 512], f32, tag="pg", bufs=1, name="bcp")
                nc.tensor.matmul(bcp[:], eb_sb[:], rcp[:], start=True, stop=True)
                nc.vector.tensor_tensor(
                    alt_t[tt][:, sl], alt_t[tt][:, sl], bcp[:], op=MULT
                )
                nc.sync.dma_start(
                    out=cin[qb][96 * tt : 96 * tt + 96, :], in_=alt_t[tt][:, sl]
                )

            pending_norm = []

            def emit_normalize(qb):
                while pending_norm:
                    normalize_half(*pending_norm.pop(0))
                nc.gpsimd.collective_compute(
                    "AllGather",
                    mybir.AluOpType.bypass,
                    replica_groups=rg,
                    ins=[cin[qb].opt()],
                    outs=[cout[qb].opt()],
                )

            def emit_vu(qb):
                # deferred one full query block behind the AllGather that
                # feeds it, so the PE queue never stalls on the collective
                sl = slice(512 * qb, 512 * qb + 512)
                alf = [
                    act.tile([128, 512], f32r, tag="alf", bufs=6, name=f"alf{k}")
                    for k in range(3)
                ]
                for k in range(3):
                    nc.sync.dma_start(
                        out=alf[k][:], in_=cout[qb][128 * k : 128 * k + 128, :]
                    )
                for m in range(3):
                    pt = ps.tile([128, 512], f32, tag="sc", bufs=2, name="vup",
                                 padded_shape=[128, 1024])
                    for k in range(3):
                        nc.tensor.matmul(
                            pt[:],
                            wvu_sb[k][:, 128 * m : 128 * m + 128],
                            alf[k][:],
                            start=(k == 0),
                            stop=(k == 2),
                        )
                    asb = work.tile([128, 512], bf16, tag="aosb", bufs=2, name="asb")
                    nc.scalar.activation(asb[:], pt[:], AF.Copy, bias=0.0)
                    nc.sync.dma_start(out=aot[128 * m : 128 * m + 128, sl], in_=asb[:])

            # schedule: projections upfront (dense full-array work holds the
            # clock gate at K=8); per-qb vl prefetch batches double as promote
            # bursts; each vu is deferred one query block behind its AllGather
            for nb in range(4):
                qk_proj(nb)
            for i in range(4):
                vl_proj(i)
            emit_attention(0)
            for i in range(4, 8):
                vl_proj(i)
            emit_normalize(0)
            emit_attention(1)
            for i in range(8, 12):
                vl_proj(i)
            emit_vu(0)
            emit_normalize(1)
            emit_attention(2)
            for i in range(12, 16):
                vl_proj(i)
            emit_vu(1)
            emit_normalize(2)
            emit_attention(3)
            emit_vu(2)
            emit_normalize(3)

            # ---- heater covers the last AllGather + vu drain --------------------
            heat(40, qlT[0])
            emit_vu(3)
            heat(16, qlT[0])

            # ---- scramble (flat reinterpretation) + transposes ------------------
            wo_sb = []
            for k in range(6):
                wt = const.tile([128, HID], bf16, name=f"wo{k}", tag="wo", bufs=6,
                                padded_shape=[128, HID])
                nc.sync.dma_start(out=wt[:], in_=WO[128 * k : 128 * k + 128, :])
                wo_sb.append(wt)

            aot_v = aot[:, :].rearrange("a b -> (a b)").rearrange(
                "(c d) -> c d", c=1024
            )
            # aotT[j][nbu] : scrambled-feature rows 128j..+128, token cols
            # 512*nbu..+512 -- split per nbu so the out-proj can start on the
            # first half while the second half is still transposing
            aotT = [
                [
                    act.tile([128, 512], bf16, name=f"aotT{j}_{nbu}", tag="aotT",
                             bufs=12)
                    for nbu in range(2)
                ]
                for j in range(6)
            ]
            for u in range(8):
                at = work.tile([128, HID], bf16, tag="at", bufs=3, name="at")
                nc.sync.dma_start(out=at[:], in_=aot_v[128 * u : 128 * u + 128, :])
                for j in range(6):
                    ptag = "sc" if j % 2 == 0 else "av"
                    tp = ps.tile([128, 128], bf16, tag=ptag,
                                 bufs=(2 if ptag == "sc" else 3), name="tpp")
                    nc.tensor.transpose(tp[:], at[:, 128 * j : 128 * j + 128], idn_sb[:])
                    dst = aotT[j][u // 4][:, 128 * (u % 4) : 128 * (u % 4) + 128]
                    if j % 2 == 0:
                        nc.vector.tensor_copy(dst, tp[:])
                    else:
                        nc.scalar.activation(dst, tp[:], AF.Copy, bias=0.0)

            # ---- output projection ---------------------------------------------
            heat(16, qlT[0])
            for nb in range(2):
                for m in range(6):
                    pt = ps.tile([128, 512], f32, tag="sc", bufs=2, name="outp",
                                 padded_shape=[128, 1024])
                    for k in range(6):
                        nc.tensor.matmul(
                            pt[:],
                            wo_sb[k][:, 128 * m : 128 * m + 128],
                            aotT[k][nb][:],
                            start=(k == 0),
                            stop=(k == 5),
                        )
                    osb = work.tile([128, 512], f32, tag="osb", bufs=2, name="osb")
                    nc.scalar.activation(
                        osb[:], pt[:], AF.Identity, bias=bo_sb[:, m : m + 1]
                    )
                    nc.sync.dma_start(
                        out=OUTT[128 * m : 128 * m + 128, 512 * nb : 512 * nb + 512],
                        in_=osb[:],
                    )
            # cover the final output DMA drain
            heat(12, qlT[0])

    nc.finalize()
    _CACHE["nc"] = nc
    return nc


def _host_prep(inputs):
    x = np.asarray(inputs["x"], np.float32)
    Wq = np.asarray(inputs["Wq"], np.float32)
    bq = np.asarray(inputs["bq"], np.float32)
    Wkd = np.asarray(inputs["Wkd"], np.float32)
    bkd = np.asarray(inputs["bkd"], np.float32)
    Wvd = np.asarray(inputs["Wvd"], np.float32)
    bvd = np.asarray(inputs["bvd"], np.float32)
    Wvu = np.asarray(inputs["Wvu"], np.float32)
    bvu = np.asarray(inputs["bvu"], np.float32)
    Wo = np.asarray(inputs["Wo"], np.float32)
    bo = np.asarray(inputs["bo"], np.float32)

    mask = np.tril(np.ones((128, 128), np.float32)).T.astype(ml_dtypes.bfloat16)
    ident = np.eye(128, dtype=ml_dtypes.bfloat16)
    eg = np.zeros((96, 3), np.float32)
    eb = np.zeros((3, 96), np.float32)
    for g in range(3):
        eg[32 * g + 16, g] = 1.0
        eb[g, 32 * g : 32 * g + 17] = 1.0

    bo_mat = np.ascontiguousarray(bo.reshape(6, 128).T)

    per_half = []
    for p in range(2):
        wq_pack = np.zeros((HID, 256), np.float32)
        wk_pack = np.zeros((HID, 256), np.float32)
        wv_pack = np.zeros((HID + 1, 224), np.float32)
        qkb = np.zeros((128, 4), np.float32)
        for hl in range(HL):
            hg = HL * p + hl
            t, g = hl // 3, hl % 3
            cols = slice(128 * t + 32 * g, 128 * t + 32 * g + 16)
            rows = slice(32 * g, 32 * g + 16)
            wq_pack[:, cols] = Wq[:, DH * hg : DH * hg + LD]
            qkb[rows, 0 + t] = bq[DH * hg : DH * hg + LD]
            wk_pack[:, cols] = Wkd[:, LD * hg : LD * hg + LD]
            qkb[rows, 2 + t] = bkd[LD * hg : LD * hg + LD]
            c0 = 32 * hl
            wv_pack[:HID, c0 : c0 + 16] = Wvd[:, LD * hg : LD * hg + LD]
            wv_pack[HID, c0 : c0 + 16] = bvd[LD * hg : LD * hg + LD]
            wv_pack[HID, c0 + 16] = 1.0
        wvu_pack = np.zeros((384, MHALF), np.float32)
        for hp in range(H):
            wvu_pack[32 * hp : 32 * hp + 16, :] = Wvu[
                LD * hp : LD * hp + LD, MHALF * p : MHALF * p + MHALF
            ]
        wvu_pack[16, :] = bvu[MHALF * p : MHALF * p + MHALF]
        per_half.append((
            wq_pack.astype(ml_dtypes.bfloat16),
            wk_pack.astype(ml_dtypes.bfloat16),
            wv_pack.astype(ml_dtypes.bfloat16),
            qkb, wvu_pack,
        ))

    in_maps = []
    for c in range(N_CORES):
        b, p = c // 2, c % 2
        xt = np.concatenate(
            [np.ascontiguousarray(x[b].T), np.ones((1, S), np.float32)], axis=0
        ).astype(ml_dtypes.bfloat16)
        wq_pack, wk_pack, wv_pack, qkb, wvu_pack = per_half[p]
        in_maps.append(
            dict(
                xt=xt, wq=wq_pack, wk=wk_pack, wv=wv_pack, qkb=qkb,
                wvu=wvu_pack,
                wo=Wo.astype(ml_dtypes.bfloat16), bo=bo_mat, mask=mask,
                ident=ident, eg=eg, eb=eb,
            )
        )
    return in_maps


def _run(inputs, **kw):
    nc = _build_nc()
    in_maps = _host_prep(inputs)
    return run_bass_kernel_spmd(nc, in_maps, core_ids=list(range(N_CORES)), **kw)


def kernel(**inputs):
    res = _run(inputs)
    out = np.empty((B, S, HID), np.float32)
    for b in range(B):
        for p in range(2):
            out[b, 1024 * p : 1024 * p + 1024, :] = res.results[2 * b + p]["outt"].T
    return out
